# revision 1
# baseline (speedup 1.0000x reference)
"""Causal multi-head attention (B=2,T=2048,C=1024,H=16,Ca=64) on 8 trn2 cores.

Sharding: the 32 (batch, head) pairs are split across 8 cores — core c gets
batch b = c//4 and heads [4g, 4g+4) where g = c%4.  Each core computes its
heads' attention plus the partial output projection through its 256-row slice
of w_o; the host sums the 4 partials per batch.

Per-core layouts (everything keeps the contraction dim on partitions):
  xT   [8,128,2048]  x[b].T c-chunked
  wq/wk[2,8,128,128] per head-pair, per c-chunk, cols = [h0 64 | h1 64]
  wv   [8,128,256]   4 heads concatenated
  wo   [2,128,1024]  rows 256g..256g+256 of w_o, c_local-chunked
  out  [16,128,1024] partial output, t-blocked

On-chip: Q^T,K^T [128(2 heads),2048]; V natural [s,a] with a ones column
appended so the A@V matmul also emits the softmax row-sums l[t]; scores are
computed transposed (S^T[s,t]) so softmax needs no cross-partition reduction
and no max-subtraction (logits are bounded: |s*scale| < ~4).
"""

import math
import sys

import numpy as np

for _p in ("/opt/trn_rl_repo",):
    if _p not in sys.path:
        sys.path.insert(0, _p)

import concourse.bass as bass
from concourse import bacc
import concourse.mybir as mybir
from concourse.bass import ts
from concourse.tile import TileContext
from concourse.bass_utils import run_bass_kernel_spmd
from contextlib import ExitStack

F32 = mybir.dt.float32
F32R = mybir.dt.float32r
AF = mybir.ActivationFunctionType

B, T, C = 2, 2048, 1024
H, CA = 16, 64
SCALE = 1.0 / math.sqrt(CA)
NCORES = 8
HPC = 4          # heads per core
TB = T // 128    # 16 t-blocks of 128
TC = T // 512    # 4 t-chunks of 512
CK = C // 128    # 8 c-chunks




def build_nc():
    nc = bacc.Bacc()
    xT = nc.declare_dram_parameter("xT", [CK, 128, T], F32R, isOutput=False)
    wq = nc.declare_dram_parameter("wq", [2, CK, 128, 128], F32R, isOutput=False)
    wk = nc.declare_dram_parameter("wk", [2, CK, 128, 128], F32R, isOutput=False)
    wv = nc.declare_dram_parameter("wv", [CK, 128, 2 * 128], F32R, isOutput=False)
    wo = nc.declare_dram_parameter("wo", [2, 128, C], F32R, isOutput=False)
    mask_d = nc.declare_dram_parameter("mask", [128, 4, 512], F32R, isOutput=False)
    ones_d = nc.declare_dram_parameter("ones", [128, 64], F32R, isOutput=False)
    out = nc.declare_dram_parameter("out", [TB, 128, C], F32, isOutput=True)

    with TileContext(nc) as tc, ExitStack() as ctx:
        const = ctx.enter_context(tc.tile_pool(name="const", bufs=1))
        persist = ctx.enter_context(tc.tile_pool(name="persist", bufs=1))

        # 0/1 causal masks for the 4 diagonal-band shifts (S^T layout [s,t]):
        # keep (1.0) where 128*d + p <= f, else 0.  Host-computed.
        mask = const.tile([128, 4, 512], F32R)
        nc.scalar.dma_start(mask[:], mask_d[:])
        ones_sb = const.tile([128, 64], F32R)
        nc.gpsimd.dma_start(ones_sb[:], ones_d[:])
        ones1 = ones_sb[0:1, :]

        q_sb = [persist.tile([128, T], F32R, tag=f"q{p}", name=f"q{p}") for p in range(2)]
        k_sb = [persist.tile([128, T], F32R, tag=f"k{p}", name=f"k{p}") for p in range(2)]
        # V natural [s,a] per head, t-blocked, with ones column at a=64
        v_sb = persist.tile([128, HPC, TB, 65], F32R, tag="v")
        nc.sync.dma_start(
            v_sb[:, :, :, 64],
            ones_d[:].rearrange("p (h b) -> p h b", h=HPC),
        )
        y_sb = [persist.tile([128, T], F32R, tag=f"y{p}", name=f"y{p}") for p in range(2)]
        wo_sb = persist.tile([128, 2, C], F32R, tag="wo")
        for cl in range(2):
            nc.gpsimd.dma_start(wo_sb[:, cl, :], wo[cl])

        # ---------------- Phase B/C: projections ----------------
        with ExitStack() as pbc:
            xw = pbc.enter_context(tc.tile_pool(name="xw", bufs=1))
            ps_qk = pbc.enter_context(tc.tile_pool(name="ps_qk", bufs=4, space="PSUM"))
            ps_v = pbc.enter_context(tc.tile_pool(name="ps_v", bufs=3, space="PSUM"))

            xT_sb = xw.tile([128, CK, T], F32R, tag="xT")
            wq_sb = xw.tile([128, 2, CK, 128], F32R, tag="wq")
            wk_sb = xw.tile([128, 2, CK, 128], F32R, tag="wk")
            wv_sb = xw.tile([128, CK, 256], F32R, tag="wv")
            # weights for pair 0 first (first matmuls need them), x chunks
            # round-robined over issuing engines so queues run in parallel
            engs = [nc.sync, nc.scalar, nc.gpsimd]
            nc.sync.dma_start(xT_sb[:, 0, :], xT[0])
            nc.scalar.dma_start(wq_sb[:, 0, 0, :], wq[0, 0])
            for ck in range(1, CK):
                engs[ck % 3].dma_start(wq_sb[:, 0, ck, :], wq[0, ck])
            for ck in range(1, CK):
                engs[ck % 3].dma_start(xT_sb[:, ck, :], xT[ck])
            for ck in range(CK):
                engs[(ck + 1) % 3].dma_start(wk_sb[:, 0, ck, :], wk[0, ck])
                engs[(ck + 2) % 3].dma_start(wq_sb[:, 1, ck, :], wq[1, ck])
                engs[ck % 3].dma_start(wk_sb[:, 1, ck, :], wk[1, ck])
                engs[(ck + 1) % 3].dma_start(wv_sb[:, ck, :], wv[ck])

            # Q^T / K^T: [128(2 heads), T]
            for p in range(2):
                for w_s, dst in ((wq_sb, q_sb), (wk_sb, k_sb)):
                    pst = [ps_qk.tile([128, 512], F32, tag="qk", name="qkps") for _ in range(TC)]
                    for ck in range(CK):
                        for tcn in range(TC):
                            nc.tensor.matmul(
                                pst[tcn][:],
                                lhsT=(w_s[:, p, ck, :]),
                                rhs=(xT_sb[:, ck, ts(tcn, 512)]),
                                start=(ck == 0), stop=(ck == CK - 1),
                            )
                    for tcn in range(TC):
                        nc.vector.tensor_copy(dst[p][:, ts(tcn, 512)], pst[tcn][:])

            # V natural: [s(=t) blocks, 4*64]
            for tb in range(TB):
                vps = ps_v.tile([128, 256], F32, tag="v")
                for ck in range(CK):
                    nc.tensor.matmul(
                        vps[:],
                        lhsT=(xT_sb[:, ck, ts(tb, 128)]),
                        rhs=(wv_sb[:, ck, :]),
                        start=(ck == 0), stop=(ck == CK - 1),
                    )
                nc.vector.tensor_copy(
                    v_sb[:, :, tb, 0:64],
                    vps[:].rearrange("p (h a) -> p h a", h=HPC),
                )

        # ---------------- Phase D + E: attention and output projection ----
        # tcn-outer so the projection for finished t-chunks overlaps attention
        with ExitStack() as pd:
            pp = pd.enter_context(tc.tile_pool(name="pp", bufs=10))
            sm = pd.enter_context(tc.tile_pool(name="sm", bufs=4))
            ob = pd.enter_context(tc.tile_pool(name="ob", bufs=3))
            ps_s = pd.enter_context(tc.tile_pool(name="ps_s", bufs=2, space="PSUM"))
            ps_y = pd.enter_context(tc.tile_pool(name="ps_y", bufs=2, space="PSUM"))
            ps_o = pd.enter_context(tc.tile_pool(name="ps_o", bufs=2, space="PSUM"))

            def proj_block(tb):
                ot = ob.tile([128, C], F32, tag="o", name="ot")
                for cc in range(2):
                    ops_ = ps_o.tile([128, 512], F32, tag="o", name="ops")
                    for cl in range(2):
                        nc.tensor.matmul(
                            ops_[:],
                            lhsT=(y_sb[cl][:, ts(tb, 128)]),
                            rhs=(wo_sb[:, cl, ts(cc, 512)]),
                            start=(cl == 0), stop=(cl == 1),
                        )
                    nc.vector.tensor_copy(ot[:, ts(cc, 512)], ops_[:])
                nc.sync.dma_start(out[tb], ot[:])

            for tcn in range(TC):
                nsb = 4 * tcn + 4
                for p in range(2):
                    for hl in range(2):
                        h = 2 * p + hl
                        b0 = 64 * hl
                        yps = ps_y.tile([128, 512], F32, tag="y", name="yps")
                        for sb2 in range(0, nsb, 2):
                            sps = ps_s.tile([128, 1024], F32, tag="s", name="sps")
                            for j in range(2):
                                nc.tensor.matmul(
                                    sps[:, ts(j, 512)],
                                    lhsT=(k_sb[p][b0:b0 + 64, ts(sb2 + j, 128)]),
                                    rhs=(q_sb[p][b0:b0 + 64, ts(tcn, 512)]),
                                    start=True, stop=True,
                                )
                            pb = pp.tile([128, 1024], F32R, tag="pb", name="pb")
                            nc.scalar.activation(pb[:], sps[:], AF.Exp, scale=SCALE)
                            for j in range(2):
                                d = sb2 + j - 4 * tcn
                                if d >= 0:
                                    w = 128 * (d + 1)
                                    o = 512 * j
                                    nc.vector.tensor_mul(
                                        pb[:, o:o + w], pb[:, o:o + w],
                                        mask[:, d, :w])
                            for j in range(2):
                                nc.tensor.matmul(
                                    yps[0:65, :],
                                    lhsT=(v_sb[:, h, sb2 + j, :]),
                                    rhs=(pb[:, ts(j, 512)]),
                                    start=(sb2 + j == 0), stop=(sb2 + j == nsb - 1),
                                )
                        # normalize: y /= l (l = row 64 of yps)
                        lrow = sm.tile([1, 512], F32R, tag="l", name="lrow")
                        nc.vector.tensor_copy(lrow[:], yps[64:65, :])
                        bps = ps_o.tile([128, 512], F32, tag="o", name="bps")
                        nc.tensor.matmul(
                            bps[0:64, :], lhsT=(ones1[:]), rhs=(lrow[:]),
                            start=True, stop=True,
                        )
                        rb = sm.tile([64, 512], F32, tag="r", name="rb")
                        nc.vector.reciprocal(rb[:], bps[0:64, :])
                        nc.vector.tensor_mul(
                            y_sb[p][b0:b0 + 64, ts(tcn, 512)],
                            yps[0:64, :], rb[:],
                        )
                # project the 4 t-blocks of this finished chunk
                for tb in range(4 * tcn, 4 * tcn + 4):
                    proj_block(tb)

    nc.compile()
    return nc


_NC = None


def _get_nc():
    global _NC
    if _NC is None:
        _NC = build_nc()
    return _NC


def _mask_arr():
    p = np.arange(128)[:, None, None]
    d = np.arange(4)[None, :, None]
    f = np.arange(512)[None, None, :]
    return np.ascontiguousarray((128 * d + p <= f).astype(np.float32))


def make_in_maps(x, w_q, w_k, w_v, w_o):
    x = np.asarray(x, dtype=np.float32)
    w_q = np.asarray(w_q, dtype=np.float32)
    w_k = np.asarray(w_k, dtype=np.float32)
    w_v = np.asarray(w_v, dtype=np.float32)
    w_o = np.asarray(w_o, dtype=np.float32)
    in_maps = []
    for c in range(NCORES):
        b, g = c // 4, c % 4
        hs = [4 * g + i for i in range(HPC)]
        xT = np.ascontiguousarray(x[b].T).reshape(CK, 128, T)
        wq_a = np.stack([
            np.concatenate([w_q[hs[2 * p]], w_q[hs[2 * p + 1]]], axis=1).reshape(CK, 128, 128)
            for p in range(2)
        ])
        wk_a = np.stack([
            np.concatenate([w_k[hs[2 * p]], w_k[hs[2 * p + 1]]], axis=1).reshape(CK, 128, 128)
            for p in range(2)
        ])
        wv_a = np.concatenate([w_v[h] for h in hs], axis=1).reshape(CK, 128, 256)
        wo_a = w_o[256 * g:256 * (g + 1)].reshape(2, 128, C)
        in_maps.append(dict(
            mask=_mask_arr(),
            ones=np.ones((128, 64), np.float32),
            xT=np.ascontiguousarray(xT),
            wq=np.ascontiguousarray(wq_a),
            wk=np.ascontiguousarray(wk_a),
            wv=np.ascontiguousarray(wv_a),
            wo=np.ascontiguousarray(wo_a),
        ))
    return in_maps


def gather_out(results):
    acc = [np.zeros((T, C), np.float64) for _ in range(B)]
    for c in range(NCORES):
        acc[c // 4] += results[c]["out"].reshape(T, C).astype(np.float64)
    return np.stack([a.astype(np.float32) for a in acc])


def run(x, w_q, w_k, w_v, w_o, trace=False, **spmd_kwargs):
    nc = _get_nc()
    in_maps = make_in_maps(x, w_q, w_k, w_v, w_o)
    res = run_bass_kernel_spmd(nc, in_maps, list(range(NCORES)), trace=trace,
                               **spmd_kwargs)
    return gather_out(res.results), res


def kernel(x, w_q, w_k, w_v, w_o):
    out, _ = run(x, w_q, w_k, w_v, w_o)
    return out



# revision 10
# speedup vs baseline: 1.1557x; 1.1557x over previous
"""Causal multi-head attention (B=2,T=2048,C=1024,H=16,Ca=64) on 8 trn2 cores.

Sharding: the 32 (batch, head) pairs are split across 8 cores - core c gets
batch b = c//4 and heads [4g, 4g+4) where g = c%4.  Each core computes its
heads' attention plus the partial output projection through its 256-row slice
of w_o; the host sums the 4 partials per batch.

Pipeline (per core), bf16 storage + fp32 PSUM, scores matmul in fp8-e4m3
DoubleRow (2x PE rate):
  - Q/K projections emit a [128=(2 groups x 64a), j, t] layout (head parity
    j within each 64-partition group, plus a zeros slot j=2) so the per-head
    fp8 scores matmul runs in DoubleRow mode - pair (head, zeros) - at
    0.5 cycles/row.
  - Scores are computed transposed (S^T[s,t]) per 512-t chunk; exp on the
    Act engine writes bf16 P^T tiles; diagonal-block triangles are zeroed
    by 0/1 mask multiplies on DVE.
  - A@V uses P^T blocks as the matmul stationary so y lands NATURAL
    [t, a] at only 65 moving rows per (t-block, s-block); the extra ones
    column of V yields the softmax denominators l in column 64.
  - normalize = per-partition reciprocal + broadcast multiply (DVE).
  - y_nat is transposed back via PE-transposes (128 rows each) for the
    output projection, whose [128,1024] psum is staged to SBUF by the Pool
    engine and DMA'd out in fp32.
Chunk-major software pipeline: projections of chunk n+1 and the output
projection of chunk n-1 are interleaved between the 4 heads of chunk n so
the Act engine (exp) never starves.
"""

import math
import sys

import numpy as np
import ml_dtypes

for _p in ("/opt/trn_rl_repo",):
    if _p not in sys.path:
        sys.path.insert(0, _p)

import concourse.bass as bass
from concourse import bacc
import concourse.mybir as mybir
from concourse.bass import ts
from concourse.tile import TileContext
from concourse.bass_utils import run_bass_kernel_spmd
from contextlib import ExitStack

F32 = mybir.dt.float32
BF16 = mybir.dt.bfloat16
FP8 = mybir.dt.float8e4
AF = mybir.ActivationFunctionType
DR = mybir.MatmulPerfMode.DoubleRow

B, T, C = 2, 2048, 1024
H, CA = 16, 64
SCALE = 1.0 / math.sqrt(CA)
NCORES = 8
HPC = 4          # heads per core
TB = T // 128    # 16 t-blocks of 128
TC = T // 512    # 4 t-chunks of 512
CK = C // 128    # 8 c-chunks

FP8_S = True     # fp8-e4m3 DoubleRow scores matmul
QK_DT = FP8 if FP8_S else BF16


def build_nc():
    nc = bacc.Bacc()
    xT_d = nc.declare_dram_parameter("xT", [TC, 128, CK, 512], BF16, isOutput=False)
    wq_d = nc.declare_dram_parameter("wq", [128, 2, CK, 128], BF16, isOutput=False)
    wk_d = nc.declare_dram_parameter("wk", [128, 2, CK, 128], BF16, isOutput=False)
    wv_d = nc.declare_dram_parameter("wv", [128, CK, 256], BF16, isOutput=False)
    wo_d = nc.declare_dram_parameter("wo", [128, 2, C], BF16, isOutput=False)
    mask_d = nc.declare_dram_parameter("mask", [128, 4, 512], BF16, isOutput=False)
    ident_d = nc.declare_dram_parameter("ident", [128, 128], BF16, isOutput=False)
    out_d = nc.declare_dram_parameter("out", [TB, 128, C], F32, isOutput=True)

    with TileContext(nc) as tc, ExitStack() as ctx:
        const = ctx.enter_context(tc.tile_pool(name="const", bufs=1))
        persist = ctx.enter_context(tc.tile_pool(name="persist", bufs=1))
        xp = ctx.enter_context(tc.tile_pool(name="xp", bufs=2))
        pbp = ctx.enter_context(tc.tile_pool(name="pbp", bufs=10))
        ynsbp = ctx.enter_context(tc.tile_pool(name="ynsbp", bufs=2))
        ytsbp = ctx.enter_context(tc.tile_pool(name="ytsbp", bufs=2))
        obp = ctx.enter_context(tc.tile_pool(name="obp", bufs=3))
        rbp = ctx.enter_context(tc.tile_pool(name="rbp", bufs=4))
        big = ctx.enter_context(tc.tile_pool(name="big", bufs=3, space="PSUM"))
        ynp = ctx.enter_context(tc.tile_pool(name="ynp", bufs=2, space="PSUM"))

        # ---- constant loads (SP + Pool queues; wq first: first matmul needs it)
        wq_sb = const.tile([128, 2, CK, 128], BF16, tag="wq", name="wq_sb")
        nc.sync.dma_start(wq_sb[:], wq_d[:])
        wk_sb = const.tile([128, 2, CK, 128], BF16, tag="wk", name="wk_sb")
        wv_sb = const.tile([128, CK, 256], BF16, tag="wv", name="wv_sb")
        wo_sb = const.tile([128, 2, C], BF16, tag="wo", name="wo_sb")
        mask_sb = const.tile([128, 4, 512], BF16, tag="mask", name="mask_sb")
        ident_sb = const.tile([128, 128], BF16, tag="ident", name="ident_sb")

        # persistent activations; q/k: [128=(grp, a), j(head parity; 2=zeros), t]
        qs = persist.tile([128, 3, T], QK_DT, tag="q", name="qs")
        ks = persist.tile([128, 3, T], QK_DT, tag="k", name="ks")
        v65 = persist.tile([128, HPC, TB, 65], BF16, tag="v", name="v65")
        if FP8_S:
            nc.gpsimd.memset(qs[:, 2, :], 0.0)
            nc.gpsimd.memset(ks[:, 2, :], 0.0)

        xts = [None] * TC

        def load_x(tcn):
            t = xp.tile([128, CK, 512], BF16, tag="xt", name=f"xt{tcn}")
            nc.sync.dma_start(t[:, 0:4, :], xT_d[tcn, :, 0:4, :])
            nc.gpsimd.dma_start(t[:, 4:8, :], xT_d[tcn, :, 4:8, :])
            xts[tcn] = t

        load_x(0)
        nc.sync.dma_start(wk_sb[:], wk_d[:])
        nc.gpsimd.dma_start(wv_sb[:], wv_d[:])
        nc.sync.dma_start(mask_sb[:], mask_d[:])
        nc.gpsimd.dma_start(wo_sb[:], wo_d[:])
        nc.sync.dma_start(ident_sb[:], ident_d[:])
        nc.vector.memset(v65[:, :, :, 64], 1.0)

        # ---- building blocks -------------------------------------------------
        def proj_qk(tcn, w_sb, dst):
            ps = big.tile([128, 2, 512], F32, tag="big", name="qkps")
            for j in range(2):
                for ck in range(CK):
                    nc.tensor.matmul(
                        ps[:, j, :],
                        lhsT=w_sb[:, j, ck, :],
                        rhs=xts[tcn][:, ck, :],
                        start=(ck == 0), stop=(ck == CK - 1),
                    )
            nc.vector.tensor_copy(dst[:, 0:2, ts(tcn, 512)], ps[:])

        def proj_v(tcn):
            ps = big.tile([128, 4, 256], F32, tag="big", name="vps")
            for tb4 in range(4):
                for ck in range(CK):
                    nc.tensor.matmul(
                        ps[:, tb4, :],
                        lhsT=xts[tcn][:, ck, ts(tb4, 128)],
                        rhs=wv_sb[:, ck, :],
                        start=(ck == 0), stop=(ck == CK - 1),
                    )
            nc.vector.tensor_copy(
                v65[:, :, 4 * tcn:4 * tcn + 4, 0:64],
                ps[:].rearrange("p tb (h a) -> p h tb a", h=HPC),
            )

        def attn_head(tcn, h, ynsb):
            nsb = 4 * tcn + 4
            p0, p1 = 64 * (h // 2), 64 * (h // 2) + 64
            jh = h % 2
            ynat = ynp.tile([128, 4, 65], F32, tag="yn", name="ynat")
            pbs = []
            for sb2 in range(0, nsb, 2):
                sps = big.tile([128, 2, 512], F32, tag="big", name="sps")
                for jj in range(2):
                    sb = sb2 + jj
                    if FP8_S:
                        # DoubleRow pair = (head slot jh, zeros slot 2)
                        nc.tensor.matmul(
                            sps[:, jj, :],
                            lhsT=ks[p0:p1, jh:3:2 - jh, ts(sb, 128)],
                            rhs=qs[p0:p1, jh:3:2 - jh, ts(tcn, 512)],
                            start=True, stop=True, perf_mode=DR,
                        )
                    else:
                        nc.tensor.matmul(
                            sps[:, jj, :],
                            lhsT=ks[p0:p1, jh, ts(sb, 128)],
                            rhs=qs[p0:p1, jh, ts(tcn, 512)],
                            start=True, stop=True,
                        )
                pb = pbp.tile([128, 2, 512], BF16, tag="pb", name="pb")
                d0 = sb2 - 4 * tcn
                if d0 < 2:
                    nc.scalar.activation(pb[:], sps[:], AF.Exp, scale=SCALE)
                else:
                    # second diagonal pair: exp only the live tail of each block
                    for jj in range(2):
                        d = d0 + jj
                        nc.gpsimd.memset(pb[:, jj, 0:128 * d], 0.0)
                        nc.scalar.activation(
                            pb[:, jj, 128 * d:512], sps[:, jj, 128 * d:512],
                            AF.Exp, scale=SCALE,
                        )
                for jj in range(2):
                    d = sb2 + jj - 4 * tcn
                    if d >= 0:
                        if d0 < 2:
                            w = 128 * (d + 1)
                            nc.vector.tensor_mul(
                                pb[:, jj, 0:w], pb[:, jj, 0:w], mask_sb[:, d, 0:w])
                        else:
                            nc.vector.tensor_mul(
                                pb[:, jj, 128 * d:128 * (d + 1)],
                                pb[:, jj, 128 * d:128 * (d + 1)],
                                mask_sb[:, d, 128 * d:128 * (d + 1)])
                pbs.append(pb)
            # A@V: y natural, one accumulation group per t-block (sequential -
            # PSUM start zeroing is bank-granular)
            for tb4 in range(4):
                last = 4 * tcn + tb4
                for sb in range(last + 1):
                    nc.tensor.matmul(
                        ynat[:, tb4, :],
                        lhsT=pbs[sb // 2][:, sb % 2, ts(tb4, 128)],
                        rhs=v65[:, h, sb, :],
                        start=(sb == 0), stop=(sb == last),
                    )
            # normalize: y /= l (column 64), broadcast along a
            rb = rbp.tile([128, 4], F32, tag="rb", name="rb")
            nc.vector.reciprocal(rb[:], ynat[:, :, 64])
            nc.vector.tensor_mul(
                ynsb[:, :, 64 * h:64 * h + 64],
                ynat[:, :, 0:64],
                rb[:].unsqueeze(-1).broadcast_to((128, 4, 64)),
            )

        def tail_transpose(ynsb):
            ytps = big.tile([128, 1024], F32, tag="big", name="ytps")
            ytv = ytps.bitcast(BF16)[:, 0:1024].rearrange(
                "p (cl tb t) -> p cl tb t", cl=2, tb=4)
            for tb4 in range(4):
                for cl in range(2):
                    nc.tensor.transpose(
                        ytv[:, cl, tb4, :],
                        ynsb[:, tb4, ts(cl, 128)],
                        ident_sb[:],
                    )
            yt = ytsbp.tile([128, 2, 4, 128], BF16, tag="yt", name="yt")
            nc.vector.tensor_copy(yt[:], ytv[:])
            return yt

        def tail_oproj(tcn, yt, tb4):
            ops = big.tile([128, 2, 512], F32, tag="big", name="ops")
            for cc in range(2):
                for cl in range(2):
                    nc.tensor.matmul(
                        ops[:, cc, :],
                        lhsT=yt[:, cl, tb4, :],
                        rhs=wo_sb[:, cl, ts(cc, 512)],
                        start=(cl == 0), stop=(cl == 1),
                    )
            ob = obp.tile([128, C], F32, tag="ob", name="ob")
            nc.vector.tensor_copy(ob[:], ops[:].rearrange("p c f -> p (c f)"))
            eng = nc.sync if tb4 % 2 == 0 else nc.gpsimd
            eng.dma_start(out_d[4 * tcn + tb4], ob[:])

        # ---- main software pipeline -----------------------------------------
        proj_qk(0, wq_sb, qs)
        proj_qk(0, wk_sb, ks)
        proj_v(0)

        ynsbs = [None] * TC
        yts = [None] * TC
        for tcn in range(TC):
            ynsb = ynsbp.tile([128, 4, 256], BF16, tag="yn", name=f"ynsb{tcn}")
            ynsbs[tcn] = ynsb
            for h in range(HPC):
                attn_head(tcn, h, ynsb)
                if h == 0 and tcn > 0:
                    yts[tcn - 1] = tail_transpose(ynsbs[tcn - 1])
                elif h == 1:
                    if tcn < TC - 1:
                        load_x(tcn + 1)
                        proj_qk(tcn + 1, wq_sb, qs)
                    if tcn > 0:
                        tail_oproj(tcn - 1, yts[tcn - 1], 0)
                        tail_oproj(tcn - 1, yts[tcn - 1], 1)
                elif h == 2:
                    if tcn < TC - 1:
                        proj_qk(tcn + 1, wk_sb, ks)
                    if tcn > 0:
                        tail_oproj(tcn - 1, yts[tcn - 1], 2)
                        tail_oproj(tcn - 1, yts[tcn - 1], 3)
                elif h == 3 and tcn < TC - 1:
                    proj_v(tcn + 1)
        yts[TC - 1] = tail_transpose(ynsbs[TC - 1])
        for tb4 in range(4):
            tail_oproj(TC - 1, yts[TC - 1], tb4)

    nc.compile()
    return nc


_NC = None


def _get_nc():
    global _NC
    if _NC is None:
        _NC = build_nc()
    return _NC


def _mask_arr():
    p = np.arange(128)[:, None, None]
    d = np.arange(4)[None, :, None]
    f = np.arange(512)[None, None, :]
    return (128 * d + p <= f).astype(ml_dtypes.bfloat16)


def _bf16(a):
    return np.ascontiguousarray(np.asarray(a, np.float32).astype(ml_dtypes.bfloat16))


def make_in_maps(x, w_q, w_k, w_v, w_o):
    x = np.asarray(x, dtype=np.float32)
    w_q = np.asarray(w_q, dtype=np.float32)
    w_k = np.asarray(w_k, dtype=np.float32)
    w_v = np.asarray(w_v, dtype=np.float32)
    w_o = np.asarray(w_o, dtype=np.float32)
    mask = np.ascontiguousarray(_mask_arr())
    ident = np.eye(128, dtype=ml_dtypes.bfloat16)
    in_maps = []
    for c in range(NCORES):
        b, g = c // 4, c % 4
        hs = [4 * g + i for i in range(HPC)]
        # xT: [TC, 128, CK, 512] (p-major per chunk)
        xT = x[b].T.reshape(CK, 128, TC, 512).transpose(2, 1, 0, 3)

        def qk_layout(w):
            # [128, 2, CK, 128]: parity-j columns = heads (j, j+2); partition
            # group g holds head 2g+j's 64 a-columns
            per_j = []
            for j in range(2):
                cols = np.concatenate(
                    [w[hs[j]], w[hs[j + 2]]], axis=1)  # [C, 128]
                per_j.append(cols.reshape(CK, 128, 128).transpose(1, 0, 2))
            return np.stack(per_j, axis=1)  # [128, 2, CK, 128]

        wv_a = np.concatenate([w_v[h] for h in hs], axis=1)  # [C, 256]
        wv_a = wv_a.reshape(CK, 128, 256).transpose(1, 0, 2)
        wo_a = w_o[256 * g:256 * (g + 1)].reshape(2, 128, C).transpose(1, 0, 2)
        in_maps.append(dict(
            mask=mask,
            ident=ident,
            xT=_bf16(xT),
            wq=_bf16(qk_layout(w_q)),
            wk=_bf16(qk_layout(w_k)),
            wv=_bf16(wv_a),
            wo=_bf16(wo_a),
        ))
    return in_maps


def gather_out(results):
    acc = [np.zeros((T, C), np.float64) for _ in range(B)]
    for c in range(NCORES):
        acc[c // 4] += results[c]["out"].reshape(T, C).astype(np.float64)
    return np.stack([a.astype(np.float32) for a in acc])


def run(x, w_q, w_k, w_v, w_o, trace=False, **spmd_kwargs):
    nc = _get_nc()
    in_maps = make_in_maps(x, w_q, w_k, w_v, w_o)
    res = run_bass_kernel_spmd(nc, in_maps, list(range(NCORES)), trace=trace,
                               **spmd_kwargs)
    return gather_out(res.results), res


def kernel(x, w_q, w_k, w_v, w_o):
    out, _ = run(x, w_q, w_k, w_v, w_o)
    return out


# revision 25
# speedup vs baseline: 1.4171x; 1.2262x over previous
"""Causal multi-head attention (B=2,T=2048,C=1024,H=16,Ca=64) on 8 trn2 cores.

Sharding: the 32 (batch, head) pairs are split across 8 cores - core c gets
batch b = c//4 and heads [4g, 4g+4) where g = c%4.  Each core computes its
heads' attention plus the partial output projection through its 256-row slice
of w_o; the host sums the 4 partials per batch.

Pipeline (per core), bf16 storage + fp32 PSUM, scores matmul in fp8-e4m3
DoubleRow (2x PE rate):
  - Q/K projections emit a [128=(2 groups x 64a), j, t] layout (head parity
    j within each 64-partition group, plus a zeros slot j=2) so the per-head
    fp8 scores matmul runs in DoubleRow mode - pair (head, zeros) - at
    0.5 cycles/row.
  - Scores are computed transposed (S^T[s,t]) per 512-t chunk; exp on the
    Act engine writes bf16 P^T tiles; diagonal-block triangles are zeroed
    by 0/1 mask multiplies on DVE.
  - A@V uses P^T blocks as the matmul stationary so y lands NATURAL
    [t, a] at only 65 moving rows per (t-block, s-block); the extra ones
    column of V yields the softmax denominators l in column 64.
  - normalize = per-partition reciprocal + broadcast multiply (DVE).
  - y_nat is transposed back via PE-transposes (128 rows each) for the
    output projection, whose [128,1024] psum is staged to SBUF by the Pool
    engine and DMA'd out in fp32.
Chunk-major software pipeline: projections of chunk n+1 and the output
projection of chunk n-1 are interleaved between the 4 heads of chunk n so
the Act engine (exp) never starves.
"""

import math
import sys

import numpy as np
import ml_dtypes

for _p in ("/opt/trn_rl_repo",):
    if _p not in sys.path:
        sys.path.insert(0, _p)

import concourse.bass as bass
from concourse import bacc
import concourse.mybir as mybir
from concourse.bass import ts
from concourse.tile import TileContext
from concourse.bass_utils import run_bass_kernel_spmd
from contextlib import ExitStack

F32 = mybir.dt.float32
BF16 = mybir.dt.bfloat16
FP8 = mybir.dt.float8e4
AF = mybir.ActivationFunctionType
DR = mybir.MatmulPerfMode.DoubleRow

B, T, C = 2, 2048, 1024
H, CA = 16, 64
SCALE = 1.0 / math.sqrt(CA)
NCORES = 8
HPC = 4          # heads per core
TB = T // 128    # 16 t-blocks of 128
TC = T // 512    # 4 t-chunks of 512
CK = C // 128    # 8 c-chunks

FP8_S = True     # fp8-e4m3 DoubleRow scores matmul
QK_DT = FP8 if FP8_S else BF16


def build_nc():
    nc = bacc.Bacc()
    xT_d = nc.declare_dram_parameter("xT", [TC, 128, CK, 512], BF16, isOutput=False)
    wq_d = nc.declare_dram_parameter("wq", [128, 2, CK, 128], BF16, isOutput=False)
    wk_d = nc.declare_dram_parameter("wk", [128, 2, CK, 128], BF16, isOutput=False)
    wv_d = nc.declare_dram_parameter("wv", [128, CK, 256], BF16, isOutput=False)
    wo_d = nc.declare_dram_parameter("wo", [128, 2, C], BF16, isOutput=False)
    mask_d = nc.declare_dram_parameter("mask", [128, 4, 512], BF16, isOutput=False)
    ident_d = nc.declare_dram_parameter("ident", [128, 128], BF16, isOutput=False)
    out_d = nc.declare_dram_parameter("out", [TB, 128, C], F32, isOutput=True)

    with TileContext(nc) as tc, ExitStack() as ctx:
        const = ctx.enter_context(tc.tile_pool(name="const", bufs=1))
        persist = ctx.enter_context(tc.tile_pool(name="persist", bufs=1))
        xp = ctx.enter_context(tc.tile_pool(name="xp", bufs=2))
        pbp = ctx.enter_context(tc.tile_pool(name="pbp", bufs=18))
        ynsbp = ctx.enter_context(tc.tile_pool(name="ynsbp", bufs=2))
        ytsbp = ctx.enter_context(tc.tile_pool(name="ytsbp", bufs=2))
        obp = ctx.enter_context(tc.tile_pool(name="obp", bufs=3))
        rbp = ctx.enter_context(tc.tile_pool(name="rbp", bufs=4))
        big = ctx.enter_context(tc.tile_pool(name="big", bufs=3, space="PSUM"))
        ynp = ctx.enter_context(tc.tile_pool(name="ynp", bufs=2, space="PSUM"))

        # ---- constant loads; j0 weight halves + x chunk 0 first (critical path)
        wq_sb = const.tile([128, 2, CK, 128], BF16, tag="wq", name="wq_sb")
        wk_sb = const.tile([128, 2, CK, 128], BF16, tag="wk", name="wk_sb")
        wv_sb = const.tile([128, CK, 256], BF16, tag="wv", name="wv_sb")
        wo_sb = const.tile([128, 2, C], BF16, tag="wo", name="wo_sb")
        mask_sb = const.tile([128, 4, 512], BF16, tag="mask", name="mask_sb")
        ident_sb = const.tile([128, 128], BF16, tag="ident", name="ident_sb")

        # persistent activations; q/k: [128=(grp, a), j(head parity; 2=zeros), t]
        qs = persist.tile([128, 3, T], QK_DT, tag="q", name="qs")
        ks = persist.tile([128, 3, T], QK_DT, tag="k", name="ks")
        v65 = persist.tile([128, HPC, TB, 65], BF16, tag="v", name="v65")

        xts = [None] * TC

        def load_x(tcn, eng4=False):
            t = xp.tile([128, CK, 512], BF16, tag="xt", name=f"xt{tcn}")
            if eng4:
                nc.sync.dma_start(t[:, 0:2, :], xT_d[tcn, :, 0:2, :])
                nc.gpsimd.dma_start(t[:, 2:4, :], xT_d[tcn, :, 2:4, :])
                nc.scalar.dma_start(t[:, 4:8, :], xT_d[tcn, :, 4:8, :])
            else:
                nc.sync.dma_start(t[:, 0:4, :], xT_d[tcn, :, 0:4, :])
                nc.gpsimd.dma_start(t[:, 4:8, :], xT_d[tcn, :, 4:8, :])
            xts[tcn] = t

        nc.sync.dma_start(wq_sb[:, 0], wq_d[:, 0])
        nc.gpsimd.dma_start(wk_sb[:, 0], wk_d[:, 0])
        load_x(0, eng4=True)
        nc.sync.dma_start(wq_sb[:, 1], wq_d[:, 1])
        nc.gpsimd.dma_start(wk_sb[:, 1], wk_d[:, 1])
        if FP8_S:
            nc.gpsimd.memset(qs[:, 2, :], 0.0)
            nc.gpsimd.memset(ks[:, 2, :], 0.0)
        nc.sync.dma_start(wv_sb[:], wv_d[:])
        nc.gpsimd.dma_start(wo_sb[:], wo_d[:])
        nc.sync.dma_start(mask_sb[:], mask_d[:])
        nc.sync.dma_start(ident_sb[:], ident_d[:])
        nc.vector.memset(v65[:, :, :, 64], 1.0)

        # ---- filler machinery ------------------------------------------------
        # Deadline-loose PE work (projections, transposes, output projection)
        # is queued as small steps and drained a few hundred ns at a time
        # between score pairs, so the Act engine (exp) never starves behind a
        # long block of non-score PE work.  Items: (est_ns, fn) or (None, mark).
        from collections import deque
        filler = deque()
        done_marks = set()

        def drain(budget=None):
            while filler:
                est, x = filler[0]
                if est is None:
                    filler.popleft()
                    done_marks.add(x)
                    continue
                if budget is not None and budget < est:
                    return
                filler.popleft()
                x()
                if budget is not None:
                    budget -= est

        def drain_until(mark):
            while mark not in done_marks:
                est, x = filler.popleft()
                if est is None:
                    done_marks.add(x)
                else:
                    x()

        def enq_proj_qk_j(tcn, j, w_sb, dst):
            box = {}
            for ck in range(CK):
                def mm(ck=ck):
                    if ck == 0:
                        box["ps"] = big.tile([128, 512], F32, tag="big",
                                             name="qkps")
                    nc.tensor.matmul(
                        box["ps"][:],
                        lhsT=w_sb[:, j, ck, :],
                        rhs=xts[tcn][:, ck, :],
                        start=(ck == 0), stop=(ck == CK - 1),
                    )
                filler.append((215, mm))
            def cp():
                nc.vector.tensor_copy(dst[:, j, ts(tcn, 512)], box["ps"][:])
            filler.append((60, cp))

        def enq_proj_v(tcn):
            box = {}
            for tb4 in range(4):
                for ck2 in range(0, CK, 2):
                    def mm(tb4=tb4, ck2=ck2):
                        if tb4 == 0 and ck2 == 0:
                            box["ps"] = big.tile([128, 4, 256], F32, tag="big",
                                                 name="vps")
                        for ck in (ck2, ck2 + 1):
                            nc.tensor.matmul(
                                box["ps"][:, tb4, :],
                                lhsT=xts[tcn][:, ck, ts(tb4, 128)],
                                rhs=wv_sb[:, ck, :],
                                start=(ck == 0), stop=(ck == CK - 1),
                            )
                    filler.append((215, mm))
            def cp():
                nc.vector.tensor_copy(
                    v65[:, :, 4 * tcn:4 * tcn + 4, 0:64],
                    box["ps"][:].rearrange("p tb (h a) -> p h tb a", h=HPC),
                )
            filler.append((60, cp))

        def attn_scores(tcn, h):
            """S^T + exp + mask for all s-blocks of (tcn, h); returns pb tiles."""
            nsb = 4 * tcn + 4
            p0, p1 = 64 * (h // 2), 64 * (h // 2) + 64
            jh = h % 2
            pbs = []
            for sb2 in range(0, nsb, 2):
                sps = big.tile([128, 2, 512], F32, tag="big", name="sps")
                for jj in range(2):
                    sb = sb2 + jj
                    if FP8_S:
                        # DoubleRow pair = (head slot jh, zeros slot 2)
                        nc.tensor.matmul(
                            sps[:, jj, :],
                            lhsT=ks[p0:p1, jh:3:2 - jh, ts(sb, 128)],
                            rhs=qs[p0:p1, jh:3:2 - jh, ts(tcn, 512)],
                            start=True, stop=True, perf_mode=DR,
                        )
                    else:
                        nc.tensor.matmul(
                            sps[:, jj, :],
                            lhsT=ks[p0:p1, jh, ts(sb, 128)],
                            rhs=qs[p0:p1, jh, ts(tcn, 512)],
                            start=True, stop=True,
                        )
                pb = pbp.tile([128, 2, 512], BF16, tag="pb", name="pb")
                d0 = sb2 - 4 * tcn
                if d0 < 2:
                    nc.scalar.activation(pb[:], sps[:], AF.Exp, scale=SCALE)
                else:
                    # second diagonal pair: exp only the live tail of each block
                    for jj in range(2):
                        d = d0 + jj
                        nc.gpsimd.memset(pb[:, jj, 0:128 * d], 0.0)
                        nc.scalar.activation(
                            pb[:, jj, 128 * d:512], sps[:, jj, 128 * d:512],
                            AF.Exp, scale=SCALE,
                        )
                for jj in range(2):
                    d = sb2 + jj - 4 * tcn
                    if d >= 0:
                        if d0 < 2:
                            w = 128 * (d + 1)
                            nc.vector.tensor_mul(
                                pb[:, jj, 0:w], pb[:, jj, 0:w], mask_sb[:, d, 0:w])
                        else:
                            nc.vector.tensor_mul(
                                pb[:, jj, 128 * d:128 * (d + 1)],
                                pb[:, jj, 128 * d:128 * (d + 1)],
                                mask_sb[:, d, 128 * d:128 * (d + 1)])
                pbs.append(pb)
                drain(1000)
            return pbs

        def attn_av(tcn, h, pbs, ynsb):
            """A@V (y natural; sequential per-t-block accumulation groups -
            PSUM start zeroing is bank-granular) + normalize."""
            ynat = ynp.tile([128, 4, 65], F32, tag="yn", name="ynat")
            for tb4 in range(4):
                last = 4 * tcn + tb4
                for sb in range(last + 1):
                    nc.tensor.matmul(
                        ynat[:, tb4, :],
                        lhsT=pbs[sb // 2][:, sb % 2, ts(tb4, 128)],
                        rhs=v65[:, h, sb, :],
                        start=(sb == 0), stop=(sb == last),
                    )
                drain(500)
            rb = rbp.tile([128, 4], F32, tag="rb", name="rb")
            nc.vector.reciprocal(rb[:], ynat[:, :, 64])
            nc.vector.tensor_mul(
                ynsb[:, :, 64 * h:64 * h + 64],
                ynat[:, :, 0:64],
                rb[:].unsqueeze(-1).broadcast_to((128, 4, 64)),
            )

        def enq_transpose(tcn, ynsb, yt):
            box = {}
            for tb4 in range(4):
                def mm(tb4=tb4):
                    if tb4 == 0:
                        box["ps"] = big.tile([128, 1024], F32, tag="big",
                                             name="ytps")
                        box["v"] = box["ps"].bitcast(BF16)[:, 0:1024].rearrange(
                            "p (cl tb t) -> p cl tb t", cl=2, tb=4)
                    for cl in range(2):
                        nc.tensor.transpose(
                            box["v"][:, cl, tb4, :],
                            ynsb[:, tb4, ts(cl, 128)],
                            ident_sb[:],
                        )
                filler.append((110, mm))
            def cp():
                nc.vector.tensor_copy(yt[:], box["v"][:])
            filler.append((60, cp))

        def oproj_mms(ops, yt, tb4):
            for cc in range(2):
                for cl in range(2):
                    nc.tensor.matmul(
                        ops[:, cc, :],
                        lhsT=yt[:, cl, tb4, :],
                        rhs=wo_sb[:, cl, ts(cc, 512)],
                        start=(cl == 0), stop=(cl == 1),
                    )

        def oproj_out(ops, tcn, tb4, copy_eng=None):
            ob = obp.tile([128, C], F32, tag="ob", name="ob")
            if copy_eng is nc.scalar:
                nc.scalar.activation(
                    ob[:], ops[:].rearrange("p c f -> p (c f)"), AF.Copy)
            else:
                nc.vector.tensor_copy(ob[:], ops[:].rearrange("p c f -> p (c f)"))
            eng = nc.sync if tb4 % 2 == 0 else nc.gpsimd
            eng.dma_start(out_d[4 * tcn + tb4], ob[:])

        def enq_oproj(tcn, yt, tb4):
            box = {}
            for cc in range(2):
                def mm(cc=cc):
                    if cc == 0:
                        box["ps"] = big.tile([128, 2, 512], F32, tag="big",
                                             name="ops")
                    for cl in range(2):
                        nc.tensor.matmul(
                            box["ps"][:, cc, :],
                            lhsT=yt[:, cl, tb4, :],
                            rhs=wo_sb[:, cl, ts(cc, 512)],
                            start=(cl == 0), stop=(cl == 1),
                        )
                filler.append((430, mm))
            filler.append((60, lambda: oproj_out(box["ps"], tcn, tb4)))

        # ---- main software pipeline -----------------------------------------
        # Head order (0,2,1,3): heads 0/2 need only the j0 slot of q/k, so
        # attention starts right after the j0 projections of a chunk; j1
        # projections, next-chunk projections and the previous chunk's output
        # projection drip in as filler between score pairs.
        HSEQ = (0, 2, 1, 3)
        # startup: chunk-0 j0 projections issued directly (critical path)
        for w_sb, dst in ((wq_sb, qs), (wk_sb, ks)):
            ps = big.tile([128, 512], F32, tag="big", name="qkps")
            for ck in range(CK):
                nc.tensor.matmul(
                    ps[:], lhsT=w_sb[:, 0, ck, :], rhs=xts[0][:, ck, :],
                    start=(ck == 0), stop=(ck == CK - 1),
                )
            nc.vector.tensor_copy(dst[:, 0, ts(0, 512)], ps[:])

        ynsbs = [None] * TC
        yts = [None] * TC
        for tcn in range(TC):
            ynsb = ynsbp.tile([128, 4, 256], BF16, tag="yn", name=f"ynsb{tcn}")
            ynsbs[tcn] = ynsb
            if tcn > 0:
                drain_until(f"j0-{tcn}")
                yts[tcn - 1] = ytsbp.tile([128, 2, 4, 128], BF16, tag="yt",
                                          name="yt")
                enq_transpose(tcn - 1, ynsbs[tcn - 1], yts[tcn - 1])
            else:
                enq_proj_v(0)
                filler.append((None, "v-0"))
            enq_proj_qk_j(tcn, 1, wq_sb, qs)
            enq_proj_qk_j(tcn, 1, wk_sb, ks)
            filler.append((None, f"j1-{tcn}"))

            prev_h = None
            prev_pbs = None
            for k, h in enumerate(HSEQ):
                if k == 2:
                    drain_until(f"j1-{tcn}")
                pbs = attn_scores(tcn, h)
                if prev_h is not None:
                    if prev_h == HSEQ[0]:
                        drain_until(f"v-{tcn}")
                    attn_av(tcn, prev_h, prev_pbs, ynsb)
                prev_h, prev_pbs = h, pbs
                if k == 0 and tcn < TC - 1:
                    load_x(tcn + 1)
                    enq_proj_qk_j(tcn + 1, 0, wq_sb, qs)
                    enq_proj_qk_j(tcn + 1, 0, wk_sb, ks)
                    filler.append((None, f"j0-{tcn + 1}"))
                    enq_proj_v(tcn + 1)
                    filler.append((None, f"v-{tcn + 1}"))
                elif k == 1 and tcn > 0:
                    for tb4 in range(4):
                        enq_oproj(tcn - 1, yts[tcn - 1], tb4)
            attn_av(tcn, prev_h, prev_pbs, ynsb)
        drain()
        # final chunk tail, pipelined per t-block; psum->sbuf copies alternate
        # DVE/Act (Act is idle after the last exp)
        n = TC - 1
        ytps = big.tile([128, 1024], F32, tag="big", name="ytps")
        ytv = ytps.bitcast(BF16)[:, 0:1024].rearrange(
            "p (cl tb t) -> p cl tb t", cl=2, tb=4)
        yt = ytsbp.tile([128, 2, 4, 128], BF16, tag="yt", name="yt")
        yts[n] = yt
        for tb4 in range(4):
            for cl in range(2):
                nc.tensor.transpose(
                    ytv[:, cl, tb4, :], ynsbs[n][:, tb4, ts(cl, 128)],
                    ident_sb[:])
            if tb4 % 2:
                nc.scalar.activation(yt[:, :, tb4, :], ytv[:, :, tb4, :], AF.Copy)
            else:
                nc.vector.tensor_copy(yt[:, :, tb4, :], ytv[:, :, tb4, :])
        for tb4 in range(4):
            ops = big.tile([128, 2, 512], F32, tag="big", name="ops")
            oproj_mms(ops, yt, tb4)
            oproj_out(ops, n, tb4, copy_eng=nc.scalar if tb4 % 2 == 0 else None)

    nc.compile()
    return nc


_NC = None


def _get_nc():
    global _NC
    if _NC is None:
        _NC = build_nc()
    return _NC


def _mask_arr():
    p = np.arange(128)[:, None, None]
    d = np.arange(4)[None, :, None]
    f = np.arange(512)[None, None, :]
    return (128 * d + p <= f).astype(ml_dtypes.bfloat16)


def _bf16(a):
    return np.ascontiguousarray(np.asarray(a, np.float32).astype(ml_dtypes.bfloat16))


def make_in_maps(x, w_q, w_k, w_v, w_o):
    x = np.asarray(x, dtype=np.float32)
    w_q = np.asarray(w_q, dtype=np.float32)
    w_k = np.asarray(w_k, dtype=np.float32)
    w_v = np.asarray(w_v, dtype=np.float32)
    w_o = np.asarray(w_o, dtype=np.float32)
    mask = np.ascontiguousarray(_mask_arr())
    ident = np.eye(128, dtype=ml_dtypes.bfloat16)
    in_maps = []
    for c in range(NCORES):
        b, g = c // 4, c % 4
        hs = [4 * g + i for i in range(HPC)]
        # xT: [TC, 128, CK, 512] (p-major per chunk)
        xT = x[b].T.reshape(CK, 128, TC, 512).transpose(2, 1, 0, 3)

        def qk_layout(w):
            # [128, 2, CK, 128]: parity-j columns = heads (j, j+2); partition
            # group g holds head 2g+j's 64 a-columns
            per_j = []
            for j in range(2):
                cols = np.concatenate(
                    [w[hs[j]], w[hs[j + 2]]], axis=1)  # [C, 128]
                per_j.append(cols.reshape(CK, 128, 128).transpose(1, 0, 2))
            return np.stack(per_j, axis=1)  # [128, 2, CK, 128]

        wv_a = np.concatenate([w_v[h] for h in hs], axis=1)  # [C, 256]
        wv_a = wv_a.reshape(CK, 128, 256).transpose(1, 0, 2)
        wo_a = w_o[256 * g:256 * (g + 1)].reshape(2, 128, C).transpose(1, 0, 2)
        in_maps.append(dict(
            mask=mask,
            ident=ident,
            xT=_bf16(xT),
            wq=_bf16(qk_layout(w_q)),
            wk=_bf16(qk_layout(w_k)),
            wv=_bf16(wv_a),
            wo=_bf16(wo_a),
        ))
    return in_maps


def gather_out(results):
    acc = [np.zeros((T, C), np.float64) for _ in range(B)]
    for c in range(NCORES):
        acc[c // 4] += results[c]["out"].reshape(T, C).astype(np.float64)
    return np.stack([a.astype(np.float32) for a in acc])


def run(x, w_q, w_k, w_v, w_o, trace=False, **spmd_kwargs):
    nc = _get_nc()
    in_maps = make_in_maps(x, w_q, w_k, w_v, w_o)
    res = run_bass_kernel_spmd(nc, in_maps, list(range(NCORES)), trace=trace,
                               **spmd_kwargs)
    return gather_out(res.results), res


def kernel(x, w_q, w_k, w_v, w_o):
    out, _ = run(x, w_q, w_k, w_v, w_o)
    return out


# revision 35
# speedup vs baseline: 1.4795x; 1.0441x over previous
"""Causal multi-head attention (B=2,T=2048,C=1024,H=16,Ca=64) on 8 trn2 cores.

Sharding: the 32 (batch, head) pairs are split across 8 cores - core c gets
batch b = c//4 and heads [4g, 4g+4) where g = c%4.  Each core computes its
heads' attention plus the partial output projection through its 256-row slice
of w_o; the host sums the 4 partials per batch.

Pipeline (per core), bf16 storage + fp32 PSUM, scores matmul in fp8-e4m3
DoubleRow (2x PE rate):
  - Q/K projections emit a [128=(2 groups x 64a), j, t] layout (head parity
    j within each 64-partition group, plus a zeros slot j=2) so the per-head
    fp8 scores matmul runs in DoubleRow mode - pair (head, zeros) - at
    0.5 cycles/row.
  - Scores are computed transposed (S^T[s,t]) per 512-t chunk; exp on the
    Act engine writes bf16 P^T tiles; diagonal-block triangles are zeroed
    by 0/1 mask multiplies on DVE.
  - A@V uses P^T blocks as the matmul stationary so y lands NATURAL
    [t, a] at only 65 moving rows per (t-block, s-block); the extra ones
    column of V yields the softmax denominators l in column 64.
  - normalize = per-partition reciprocal + broadcast multiply (DVE).
  - y_nat is transposed back via PE-transposes (128 rows each) for the
    output projection, whose [128,1024] psum is staged to SBUF by the Pool
    engine and DMA'd out in fp32.
Chunk-major software pipeline: projections of chunk n+1 and the output
projection of chunk n-1 are interleaved between the 4 heads of chunk n so
the Act engine (exp) never starves.
"""

import math
import sys

import numpy as np
import ml_dtypes

for _p in ("/opt/trn_rl_repo",):
    if _p not in sys.path:
        sys.path.insert(0, _p)

import concourse.bass as bass
from concourse import bacc
import concourse.mybir as mybir
from concourse.bass import ts
from concourse.tile import TileContext
from concourse.bass_utils import run_bass_kernel_spmd
from contextlib import ExitStack

F32 = mybir.dt.float32
BF16 = mybir.dt.bfloat16
FP8 = mybir.dt.float8e4
AF = mybir.ActivationFunctionType
DR = mybir.MatmulPerfMode.DoubleRow

B, T, C = 2, 2048, 1024
H, CA = 16, 64
SCALE = 1.0 / math.sqrt(CA)
NCORES = 8
HPC = 4          # heads per core
TB = T // 128    # 16 t-blocks of 128
TC = T // 512    # 4 t-chunks of 512
CK = C // 128    # 8 c-chunks

FP8_S = True     # fp8-e4m3 DoubleRow scores matmul
QK_DT = FP8 if FP8_S else BF16


def build_nc():
    nc = bacc.Bacc()
    xT_d = nc.declare_dram_parameter("xT", [TC, 128, CK, 512], BF16, isOutput=False)
    wq_d = nc.declare_dram_parameter("wq", [128, 2, CK, 128], BF16, isOutput=False)
    wk_d = nc.declare_dram_parameter("wk", [128, 2, CK, 128], BF16, isOutput=False)
    wv_d = nc.declare_dram_parameter("wv", [128, CK, 256], BF16, isOutput=False)
    wo_d = nc.declare_dram_parameter("wo", [128, 2, C], BF16, isOutput=False)
    mask_d = nc.declare_dram_parameter("mask", [128, 4, 512], BF16, isOutput=False)
    ident_d = nc.declare_dram_parameter("ident", [128, 128], BF16, isOutput=False)
    out_d = nc.declare_dram_parameter("out", [TB, 128, C], F32, isOutput=True)

    with TileContext(nc) as tc, ExitStack() as ctx:
        const = ctx.enter_context(tc.tile_pool(name="const", bufs=1))
        persist = ctx.enter_context(tc.tile_pool(name="persist", bufs=1))
        xp = ctx.enter_context(tc.tile_pool(name="xp", bufs=2))
        pbp = ctx.enter_context(tc.tile_pool(name="pbp", bufs=18))
        ynsbp = ctx.enter_context(tc.tile_pool(name="ynsbp", bufs=2))
        ytsbp = ctx.enter_context(tc.tile_pool(name="ytsbp", bufs=4))
        obp = ctx.enter_context(tc.tile_pool(name="obp", bufs=4))
        rbp = ctx.enter_context(tc.tile_pool(name="rbp", bufs=4))
        big = ctx.enter_context(tc.tile_pool(name="big", bufs=3, space="PSUM"))
        ynp = ctx.enter_context(tc.tile_pool(name="ynp", bufs=2, space="PSUM"))

        # ---- constant loads; j0 weight halves + x chunk 0 first (critical path)
        wq_sb = const.tile([128, 2, CK, 128], BF16, tag="wq", name="wq_sb")
        wk_sb = const.tile([128, 2, CK, 128], BF16, tag="wk", name="wk_sb")
        wv_sb = const.tile([128, CK, 256], BF16, tag="wv", name="wv_sb")
        wo_sb = const.tile([128, 2, C], BF16, tag="wo", name="wo_sb")
        mask_sb = const.tile([128, 4, 512], BF16, tag="mask", name="mask_sb")
        ident_sb = const.tile([128, 128], BF16, tag="ident", name="ident_sb")

        # persistent activations; q/k: [128=(grp, a), j(head parity; 2=zeros), t]
        qs = persist.tile([128, 3, T], QK_DT, tag="q", name="qs")
        ks = persist.tile([128, 3, T], QK_DT, tag="k", name="ks")
        v65 = persist.tile([128, HPC, TB, 65], BF16, tag="v", name="v65")

        xts = [None] * TC

        def load_x(tcn, eng4=False):
            t = xp.tile([128, CK, 512], BF16, tag="xt", name=f"xt{tcn}")
            if eng4:
                nc.sync.dma_start(t[:, 0:2, :], xT_d[tcn, :, 0:2, :])
                nc.gpsimd.dma_start(t[:, 2:4, :], xT_d[tcn, :, 2:4, :])
                nc.scalar.dma_start(t[:, 4:8, :], xT_d[tcn, :, 4:8, :])
            else:
                nc.sync.dma_start(t[:, 0:4, :], xT_d[tcn, :, 0:4, :])
                nc.gpsimd.dma_start(t[:, 4:8, :], xT_d[tcn, :, 4:8, :])
            xts[tcn] = t

        nc.sync.dma_start(wq_sb[:, 0], wq_d[:, 0])
        nc.gpsimd.dma_start(wk_sb[:, 0], wk_d[:, 0])
        load_x(0, eng4=True)
        nc.sync.dma_start(wq_sb[:, 1], wq_d[:, 1])
        nc.gpsimd.dma_start(wk_sb[:, 1], wk_d[:, 1])
        if FP8_S:
            nc.gpsimd.memset(qs[:, 2, :], 0.0)
            nc.gpsimd.memset(ks[:, 2, :], 0.0)
        nc.sync.dma_start(wv_sb[:], wv_d[:])
        nc.gpsimd.dma_start(wo_sb[:], wo_d[:])
        nc.sync.dma_start(mask_sb[:], mask_d[:])
        nc.sync.dma_start(ident_sb[:], ident_d[:])
        nc.vector.memset(v65[:, :, :, 64], 1.0)

        # ---- filler machinery ------------------------------------------------
        # Deadline-loose PE work (projections, transposes, output projection)
        # is queued as small steps and drained a few hundred ns at a time
        # between score pairs, so the Act engine (exp) never starves behind a
        # long block of non-score PE work.  Items: (est_ns, fn) or (None, mark).
        from collections import deque
        filler = deque()
        done_marks = set()

        def drain(budget=None):
            while filler:
                est, x = filler[0]
                if est is None:
                    filler.popleft()
                    done_marks.add(x)
                    continue
                if budget is not None and budget < est:
                    return
                filler.popleft()
                x()
                if budget is not None:
                    budget -= est

        def drain_until(mark):
            while mark not in done_marks:
                est, x = filler.popleft()
                if est is None:
                    done_marks.add(x)
                else:
                    x()

        def enq_proj_qk_j(tcn, j, w_sb, dst):
            box = {}
            for ck in range(CK):
                def mm(ck=ck):
                    if ck == 0:
                        box["ps"] = big.tile([128, 512], F32, tag="big",
                                             name="qkps")
                    nc.tensor.matmul(
                        box["ps"][:],
                        lhsT=w_sb[:, j, ck, :],
                        rhs=xts[tcn][:, ck, :],
                        start=(ck == 0), stop=(ck == CK - 1),
                    )
                filler.append((215, mm))
            def cp():
                nc.vector.tensor_copy(dst[:, j, ts(tcn, 512)], box["ps"][:])
            filler.append((60, cp))

        def enq_proj_v(tcn):
            box = {}
            for tb4 in range(4):
                for ck2 in range(0, CK, 2):
                    def mm(tb4=tb4, ck2=ck2):
                        if tb4 == 0 and ck2 == 0:
                            box["ps"] = big.tile([128, 4, 256], F32, tag="big",
                                                 name="vps")
                        for ck in (ck2, ck2 + 1):
                            nc.tensor.matmul(
                                box["ps"][:, tb4, :],
                                lhsT=xts[tcn][:, ck, ts(tb4, 128)],
                                rhs=wv_sb[:, ck, :],
                                start=(ck == 0), stop=(ck == CK - 1),
                            )
                    filler.append((215, mm))
            def cp():
                nc.vector.tensor_copy(
                    v65[:, :, 4 * tcn:4 * tcn + 4, 0:64],
                    box["ps"][:].rearrange("p tb (h a) -> p h tb a", h=HPC),
                )
            filler.append((60, cp))

        def attn_scores(tcn, h):
            """S^T + exp + mask for all s-blocks of (tcn, h); returns pb tiles."""
            nsb = 4 * tcn + 4
            p0, p1 = 64 * (h // 2), 64 * (h // 2) + 64
            jh = h % 2
            pbs = []
            for sb2 in range(0, nsb, 2):
                sps = big.tile([128, 2, 512], F32, tag="big", name="sps")
                for jj in range(2):
                    sb = sb2 + jj
                    if FP8_S:
                        # DoubleRow pair = (head slot jh, zeros slot 2)
                        nc.tensor.matmul(
                            sps[:, jj, :],
                            lhsT=ks[p0:p1, jh:3:2 - jh, ts(sb, 128)],
                            rhs=qs[p0:p1, jh:3:2 - jh, ts(tcn, 512)],
                            start=True, stop=True, perf_mode=DR,
                        )
                    else:
                        nc.tensor.matmul(
                            sps[:, jj, :],
                            lhsT=ks[p0:p1, jh, ts(sb, 128)],
                            rhs=qs[p0:p1, jh, ts(tcn, 512)],
                            start=True, stop=True,
                        )
                pb = pbp.tile([128, 2, 512], BF16, tag="pb", name="pb")
                d0 = sb2 - 4 * tcn
                if d0 < 2:
                    nc.scalar.activation(pb[:], sps[:], AF.Exp, scale=SCALE)
                else:
                    # second diagonal pair: exp only the live tail of each block
                    for jj in range(2):
                        d = d0 + jj
                        nc.gpsimd.memset(pb[:, jj, 0:128 * d], 0.0)
                        nc.scalar.activation(
                            pb[:, jj, 128 * d:512], sps[:, jj, 128 * d:512],
                            AF.Exp, scale=SCALE,
                        )
                for jj in range(2):
                    d = sb2 + jj - 4 * tcn
                    if d >= 0:
                        if d0 < 2:
                            w = 128 * (d + 1)
                            nc.vector.tensor_mul(
                                pb[:, jj, 0:w], pb[:, jj, 0:w], mask_sb[:, d, 0:w])
                        else:
                            nc.vector.tensor_mul(
                                pb[:, jj, 128 * d:128 * (d + 1)],
                                pb[:, jj, 128 * d:128 * (d + 1)],
                                mask_sb[:, d, 128 * d:128 * (d + 1)])
                pbs.append(pb)
                drain(700)
            return pbs

        def attn_av(tcn, h, pbs, ynsb):
            """A@V (y natural; sequential per-t-block accumulation groups -
            PSUM start zeroing is bank-granular) + normalize."""
            ynat = ynp.tile([128, 4, 65], F32, tag="yn", name="ynat")
            for tb4 in range(4):
                last = 4 * tcn + tb4
                for sb in range(last + 1):
                    nc.tensor.matmul(
                        ynat[:, tb4, :],
                        lhsT=pbs[sb // 2][:, sb % 2, ts(tb4, 128)],
                        rhs=v65[:, h, sb, :],
                        start=(sb == 0), stop=(sb == last),
                    )
            rb = rbp.tile([128, 4], F32, tag="rb", name="rb")
            nc.vector.reciprocal(rb[:], ynat[:, :, 64])
            nc.vector.tensor_mul(
                ynsb[:, :, 64 * h:64 * h + 64],
                ynat[:, :, 0:64],
                rb[:].unsqueeze(-1).broadcast_to((128, 4, 64)),
            )

        def enq_transpose(tcn, ynsb, yt):
            box = {}
            for tb4 in range(4):
                def mm(tb4=tb4):
                    if tb4 == 0:
                        box["ps"] = big.tile([128, 1024], F32, tag="big",
                                             name="ytps")
                        box["v"] = box["ps"].bitcast(BF16)[:, 0:1024].rearrange(
                            "p (cl tb t) -> p cl tb t", cl=2, tb=4)
                    for cl in range(2):
                        nc.tensor.transpose(
                            box["v"][:, cl, tb4, :],
                            ynsb[:, tb4, ts(cl, 128)],
                            ident_sb[:],
                        )
                filler.append((110, mm))
            def cp():
                nc.vector.tensor_copy(yt[:], box["v"][:])
            filler.append((60, cp))

        def oproj_mms(ops, yt, tb4):
            for cc in range(2):
                for cl in range(2):
                    nc.tensor.matmul(
                        ops[:, cc, :],
                        lhsT=yt[:, cl, tb4, :],
                        rhs=wo_sb[:, cl, ts(cc, 512)],
                        start=(cl == 0), stop=(cl == 1),
                    )

        def oproj_out(ops, tcn, tb4, copy_eng=None):
            ob = obp.tile([128, C], F32, tag="ob", name="ob")
            if copy_eng is nc.scalar:
                nc.scalar.activation(
                    ob[:], ops[:].rearrange("p c f -> p (c f)"), AF.Copy)
            else:
                nc.vector.tensor_copy(ob[:], ops[:].rearrange("p c f -> p (c f)"))
            eng = nc.sync if tb4 % 2 == 0 else nc.gpsimd
            eng.dma_start(out_d[4 * tcn + tb4], ob[:])

        def enq_oproj(tcn, yt, tb4):
            box = {}
            for cc in range(2):
                def mm(cc=cc):
                    if cc == 0:
                        box["ps"] = big.tile([128, 2, 512], F32, tag="big",
                                             name="ops")
                    for cl in range(2):
                        nc.tensor.matmul(
                            box["ps"][:, cc, :],
                            lhsT=yt[:, cl, tb4, :],
                            rhs=wo_sb[:, cl, ts(cc, 512)],
                            start=(cl == 0), stop=(cl == 1),
                        )
                filler.append((430, mm))
            filler.append((60, lambda: oproj_out(box["ps"], tcn, tb4)))

        # ---- main software pipeline -----------------------------------------
        # Head order (0,2,1,3): heads 0/2 need only the j0 slot of q/k, so
        # attention starts right after the j0 projections of a chunk; j1
        # projections, next-chunk projections and the previous chunk's output
        # projection drip in as filler between score pairs.
        HSEQ = (0, 2, 1, 3)
        # startup: chunk-0 j0 projections issued directly (critical path)
        for w_sb, dst in ((wq_sb, qs), (wk_sb, ks)):
            ps = big.tile([128, 512], F32, tag="big", name="qkps")
            for ck in range(CK):
                nc.tensor.matmul(
                    ps[:], lhsT=w_sb[:, 0, ck, :], rhs=xts[0][:, ck, :],
                    start=(ck == 0), stop=(ck == CK - 1),
                )
            nc.vector.tensor_copy(dst[:, 0, ts(0, 512)], ps[:])

        ynsbs = [None] * TC
        yts = [None] * TC
        for tcn in range(TC):
            ynsb = ynsbp.tile([128, 4, 256], BF16, tag="yn", name=f"ynsb{tcn}")
            ynsbs[tcn] = ynsb
            if tcn > 0:
                drain_until(f"j0-{tcn}")
            enq_proj_qk_j(tcn, 1, wq_sb, qs)
            enq_proj_qk_j(tcn, 1, wk_sb, ks)
            filler.append((None, f"j1-{tcn}"))
            if tcn > 0:
                yts[tcn - 1] = ytsbp.tile([128, 2, 4, 128], BF16, tag="yt",
                                          name="yt")
                enq_transpose(tcn - 1, ynsbs[tcn - 1], yts[tcn - 1])
            else:
                enq_proj_v(0)
                filler.append((None, "v-0"))

            prev_h = None
            prev_pbs = None
            for k, h in enumerate(HSEQ):
                if k == 2:
                    drain_until(f"j1-{tcn}")
                pbs = attn_scores(tcn, h)
                if prev_h is not None:
                    if prev_h == HSEQ[0]:
                        drain_until(f"v-{tcn}")
                    attn_av(tcn, prev_h, prev_pbs, ynsb)
                prev_h, prev_pbs = h, pbs
                if k == 0 and tcn < TC - 1:
                    load_x(tcn + 1)
                    enq_proj_qk_j(tcn + 1, 0, wq_sb, qs)
                    enq_proj_qk_j(tcn + 1, 0, wk_sb, ks)
                    filler.append((None, f"j0-{tcn + 1}"))
                    enq_proj_v(tcn + 1)
                    filler.append((None, f"v-{tcn + 1}"))
                elif k == 1 and tcn >= 2:
                    # output projections are deferred one extra chunk into the
                    # Act-bound phase where the PE has slack
                    for m in ([0] if tcn == 2 else [1, 2]):
                        for tb4 in range(4):
                            enq_oproj(m, yts[m], tb4)
                elif k == 3 and tcn == TC - 1:
                    # pre-issue cl0 transposes of the final chunk (heads 0/1
                    # normalized by now)
                    fin_ytps = big.tile([128, 1024], F32, tag="big",
                                        name="ytps")
                    fin_ytv = fin_ytps.bitcast(BF16)[:, 0:1024].rearrange(
                        "p (cl tb t) -> p cl tb t", cl=2, tb=4)
                    for tb4 in range(4):
                        nc.tensor.transpose(
                            fin_ytv[:, 0, tb4, :], ynsb[:, tb4, 0:128],
                            ident_sb[:])
            if tcn < TC - 1:
                attn_av(tcn, prev_h, prev_pbs, ynsb)
        drain()
        # final chunk tail: per-t-block AV -> normalize -> transpose ->
        # output projection pipeline (copies alternate DVE/Act; the last
        # block's staging and DMA are split across engines/queues)
        n, h = TC - 1, HSEQ[-1]
        ynsb = ynsbs[n]
        yt = ytsbp.tile([128, 2, 4, 128], BF16, tag="yt", name="yt")
        ynat = ynp.tile([128, 4, 65], F32, tag="yn", name="ynat")
        rb = rbp.tile([128, 4], F32, tag="rb", name="rb")
        for tb4 in range(4):
            last = 4 * n + tb4
            for sb in range(last + 1):
                nc.tensor.matmul(
                    ynat[:, tb4, :],
                    lhsT=prev_pbs[sb // 2][:, sb % 2, ts(tb4, 128)],
                    rhs=v65[:, h, sb, :],
                    start=(sb == 0), stop=(sb == last),
                )
            nc.vector.reciprocal(rb[:, tb4:tb4 + 1], ynat[:, tb4, 64:65])
            nc.vector.tensor_mul(
                ynsb[:, tb4, 64 * h:64 * h + 64],
                ynat[:, tb4, 0:64],
                rb[:, tb4:tb4 + 1].broadcast_to((128, 64)),
            )
            nc.tensor.transpose(
                fin_ytv[:, 1, tb4, :], ynsb[:, tb4, ts(1, 128)], ident_sb[:])
            if tb4 % 2:
                nc.scalar.activation(yt[:, :, tb4, :], fin_ytv[:, :, tb4, :],
                                     AF.Copy)
            else:
                nc.vector.tensor_copy(yt[:, :, tb4, :], fin_ytv[:, :, tb4, :])
        for tb4 in range(4):
            ops = big.tile([128, 2, 512], F32, tag="big", name="ops")
            oproj_mms(ops, yt, tb4)
            if tb4 < 3:
                oproj_out(ops, n, tb4,
                          copy_eng=nc.scalar if tb4 % 2 == 0 else None)
            else:
                ob = obp.tile([128, C], F32, tag="ob", name="ob")
                nc.vector.tensor_copy(ob[:, 0:512], ops[:, 0, :])
                nc.scalar.activation(ob[:, 512:1024], ops[:, 1, :], AF.Copy)
                nc.sync.dma_start(out_d[4 * n + tb4, :, 0:512], ob[:, 0:512])
                nc.gpsimd.dma_start(out_d[4 * n + tb4, :, 512:1024],
                                    ob[:, 512:1024])

    nc.compile()
    return nc


_NC = None


def _get_nc():
    global _NC
    if _NC is None:
        _NC = build_nc()
    return _NC


def _mask_arr():
    p = np.arange(128)[:, None, None]
    d = np.arange(4)[None, :, None]
    f = np.arange(512)[None, None, :]
    return (128 * d + p <= f).astype(ml_dtypes.bfloat16)


def _bf16(a):
    return np.ascontiguousarray(np.asarray(a, np.float32).astype(ml_dtypes.bfloat16))


def make_in_maps(x, w_q, w_k, w_v, w_o):
    x = np.asarray(x, dtype=np.float32)
    w_q = np.asarray(w_q, dtype=np.float32)
    w_k = np.asarray(w_k, dtype=np.float32)
    w_v = np.asarray(w_v, dtype=np.float32)
    w_o = np.asarray(w_o, dtype=np.float32)
    mask = np.ascontiguousarray(_mask_arr())
    ident = np.eye(128, dtype=ml_dtypes.bfloat16)
    in_maps = []
    for c in range(NCORES):
        b, g = c // 4, c % 4
        hs = [4 * g + i for i in range(HPC)]
        # xT: [TC, 128, CK, 512] (p-major per chunk)
        xT = x[b].T.reshape(CK, 128, TC, 512).transpose(2, 1, 0, 3)

        def qk_layout(w):
            # [128, 2, CK, 128]: parity-j columns = heads (j, j+2); partition
            # group g holds head 2g+j's 64 a-columns
            per_j = []
            for j in range(2):
                cols = np.concatenate(
                    [w[hs[j]], w[hs[j + 2]]], axis=1)  # [C, 128]
                per_j.append(cols.reshape(CK, 128, 128).transpose(1, 0, 2))
            return np.stack(per_j, axis=1)  # [128, 2, CK, 128]

        wv_a = np.concatenate([w_v[h] for h in hs], axis=1)  # [C, 256]
        wv_a = wv_a.reshape(CK, 128, 256).transpose(1, 0, 2)
        wo_a = w_o[256 * g:256 * (g + 1)].reshape(2, 128, C).transpose(1, 0, 2)
        in_maps.append(dict(
            mask=mask,
            ident=ident,
            xT=_bf16(xT),
            wq=_bf16(qk_layout(w_q)),
            wk=_bf16(qk_layout(w_k)),
            wv=_bf16(wv_a),
            wo=_bf16(wo_a),
        ))
    return in_maps


def gather_out(results):
    acc = [np.zeros((T, C), np.float64) for _ in range(B)]
    for c in range(NCORES):
        acc[c // 4] += results[c]["out"].reshape(T, C).astype(np.float64)
    return np.stack([a.astype(np.float32) for a in acc])


def run(x, w_q, w_k, w_v, w_o, trace=False, **spmd_kwargs):
    nc = _get_nc()
    in_maps = make_in_maps(x, w_q, w_k, w_v, w_o)
    res = run_bass_kernel_spmd(nc, in_maps, list(range(NCORES)), trace=trace,
                               **spmd_kwargs)
    return gather_out(res.results), res


def kernel(x, w_q, w_k, w_v, w_o):
    out, _ = run(x, w_q, w_k, w_v, w_o)
    return out


# revision 44
# speedup vs baseline: 1.4930x; 1.0091x over previous
"""Causal multi-head attention (B=2,T=2048,C=1024,H=16,Ca=64) on 8 trn2 cores.

Sharding: the 32 (batch, head) pairs are split across 8 cores - core c gets
batch b = c//4 and heads [4g, 4g+4) where g = c%4.  Each core computes its
heads' attention plus the partial output projection through its 256-row slice
of w_o; the host sums the 4 partials per batch.

Pipeline (per core), bf16 storage + fp32 PSUM, scores matmul in fp8-e4m3
DoubleRow (2x PE rate):
  - Q/K projections emit a [128=(2 groups x 64a), j, t] layout (head parity
    j within each 64-partition group, plus a zeros slot j=2) so the per-head
    fp8 scores matmul runs in DoubleRow mode - pair (head, zeros) - at
    0.5 cycles/row.
  - Scores are computed transposed (S^T[s,t]) per 512-t chunk; exp on the
    Act engine writes bf16 P^T tiles; diagonal-block triangles are zeroed
    by 0/1 mask multiplies on DVE.
  - A@V uses P^T blocks as the matmul stationary so y lands NATURAL
    [t, a] at only 65 moving rows per (t-block, s-block); the extra ones
    column of V yields the softmax denominators l in column 64.
  - normalize = per-partition reciprocal + broadcast multiply (DVE).
  - y_nat is transposed back via PE-transposes (128 rows each) for the
    output projection, whose [128,1024] psum is staged to SBUF by the Pool
    engine and DMA'd out in fp32.
Chunk-major software pipeline: projections of chunk n+1 and the output
projection of chunk n-1 are interleaved between the 4 heads of chunk n so
the Act engine (exp) never starves.
"""

import math
import sys

import numpy as np
import ml_dtypes

for _p in ("/opt/trn_rl_repo",):
    if _p not in sys.path:
        sys.path.insert(0, _p)

import concourse.bass as bass
from concourse import bacc
import concourse.mybir as mybir
from concourse.bass import ts
from concourse.tile import TileContext
from concourse.bass_utils import run_bass_kernel_spmd
from contextlib import ExitStack

F32 = mybir.dt.float32
BF16 = mybir.dt.bfloat16
FP8 = mybir.dt.float8e4
AF = mybir.ActivationFunctionType
DR = mybir.MatmulPerfMode.DoubleRow

B, T, C = 2, 2048, 1024
H, CA = 16, 64
SCALE = 1.0 / math.sqrt(CA)
NCORES = 8
HPC = 4          # heads per core
TB = T // 128    # 16 t-blocks of 128
TC = T // 512    # 4 t-chunks of 512
CK = C // 128    # 8 c-chunks

FP8_S = True     # fp8-e4m3 DoubleRow scores matmul
QK_DT = FP8 if FP8_S else BF16


def build_nc():
    nc = bacc.Bacc()
    xT_d = nc.declare_dram_parameter("xT", [TC, 128, CK, 512], BF16, isOutput=False)
    wq_d = nc.declare_dram_parameter("wq", [128, 2, CK, 128], BF16, isOutput=False)
    wk_d = nc.declare_dram_parameter("wk", [128, 2, CK, 128], BF16, isOutput=False)
    wv_d = nc.declare_dram_parameter("wv", [128, CK, 256], BF16, isOutput=False)
    wo_d = nc.declare_dram_parameter("wo", [128, 2, C], BF16, isOutput=False)
    mask_d = nc.declare_dram_parameter("mask", [128, 4, 512], BF16, isOutput=False)
    ident_d = nc.declare_dram_parameter("ident", [128, 128], BF16, isOutput=False)
    out_d = nc.declare_dram_parameter("out", [TB, 128, C], F32, isOutput=True)

    with TileContext(nc) as tc, ExitStack() as ctx:
        const = ctx.enter_context(tc.tile_pool(name="const", bufs=1))
        persist = ctx.enter_context(tc.tile_pool(name="persist", bufs=1))
        xp = ctx.enter_context(tc.tile_pool(name="xp", bufs=2))
        pbp = ctx.enter_context(tc.tile_pool(name="pbp", bufs=28))
        ynsbp = ctx.enter_context(tc.tile_pool(name="ynsbp", bufs=2))
        ytsbp = ctx.enter_context(tc.tile_pool(name="ytsbp", bufs=4))
        obp = ctx.enter_context(tc.tile_pool(name="obp", bufs=4))
        rbp = ctx.enter_context(tc.tile_pool(name="rbp", bufs=4))
        big = ctx.enter_context(tc.tile_pool(name="big", bufs=3, space="PSUM"))
        ynp = ctx.enter_context(tc.tile_pool(name="ynp", bufs=2, space="PSUM"))

        # ---- constant loads; j0 weight halves + x chunk 0 first (critical path)
        wq_sb = const.tile([128, 2, CK, 128], BF16, tag="wq", name="wq_sb")
        wk_sb = const.tile([128, 2, CK, 128], BF16, tag="wk", name="wk_sb")
        wv_sb = const.tile([128, CK, 256], BF16, tag="wv", name="wv_sb")
        wo_sb = const.tile([128, 2, C], BF16, tag="wo", name="wo_sb")
        mask_sb = const.tile([128, 4, 512], BF16, tag="mask", name="mask_sb")
        ident_sb = const.tile([128, 128], BF16, tag="ident", name="ident_sb")

        # persistent activations; q/k: [128=(grp, a), j(head parity; 2=zeros), t]
        qs = persist.tile([128, 3, T], QK_DT, tag="q", name="qs")
        ks = persist.tile([128, 3, T], QK_DT, tag="k", name="ks")
        v65 = persist.tile([128, HPC, TB, 65], BF16, tag="v", name="v65")

        xts = [None] * TC

        def load_x(tcn, eng4=False):
            t = xp.tile([128, CK, 512], BF16, tag="xt", name=f"xt{tcn}")
            if eng4:
                nc.sync.dma_start(t[:, 0:2, :], xT_d[tcn, :, 0:2, :])
                nc.gpsimd.dma_start(t[:, 2:4, :], xT_d[tcn, :, 2:4, :])
                nc.scalar.dma_start(t[:, 4:8, :], xT_d[tcn, :, 4:8, :])
            else:
                nc.sync.dma_start(t[:, 0:4, :], xT_d[tcn, :, 0:4, :])
                nc.gpsimd.dma_start(t[:, 4:8, :], xT_d[tcn, :, 4:8, :])
            xts[tcn] = t

        nc.sync.dma_start(wq_sb[:, 0], wq_d[:, 0])
        nc.gpsimd.dma_start(wk_sb[:, 0], wk_d[:, 0])
        load_x(0, eng4=True)
        nc.sync.dma_start(wq_sb[:, 1], wq_d[:, 1])
        nc.gpsimd.dma_start(wk_sb[:, 1], wk_d[:, 1])
        if FP8_S:
            nc.gpsimd.memset(qs[:, 2, :], 0.0)
            nc.gpsimd.memset(ks[:, 2, :], 0.0)
        nc.sync.dma_start(wv_sb[:], wv_d[:])
        nc.gpsimd.dma_start(wo_sb[:], wo_d[:])
        nc.sync.dma_start(mask_sb[:], mask_d[:])
        nc.sync.dma_start(ident_sb[:], ident_d[:])
        nc.vector.memset(v65[:, :, :, 64], 1.0)

        # ---- filler machinery ------------------------------------------------
        # Deadline-loose PE work (projections, transposes, output projection)
        # is queued as small steps and drained a few hundred ns at a time
        # between score pairs, so the Act engine (exp) never starves behind a
        # long block of non-score PE work.  Items: (est_ns, fn) or (None, mark).
        from collections import deque
        filler = deque()
        done_marks = set()

        def drain(budget=None):
            while filler:
                est, x = filler[0]
                if est is None:
                    filler.popleft()
                    done_marks.add(x)
                    continue
                if budget is not None and budget < est:
                    return
                filler.popleft()
                x()
                if budget is not None:
                    budget -= est

        def drain_until(mark):
            while mark not in done_marks:
                est, x = filler.popleft()
                if est is None:
                    done_marks.add(x)
                else:
                    x()

        def enq_proj_qk_j(tcn, j, w_sb, dst):
            box = {}
            for ck in range(CK):
                def mm(ck=ck):
                    if ck == 0:
                        box["ps"] = big.tile([128, 512], F32, tag="big",
                                             name="qkps")
                    nc.tensor.matmul(
                        box["ps"][:],
                        lhsT=w_sb[:, j, ck, :],
                        rhs=xts[tcn][:, ck, :],
                        start=(ck == 0), stop=(ck == CK - 1),
                    )
                filler.append((215, mm))
            def cp():
                nc.vector.tensor_copy(dst[:, j, ts(tcn, 512)], box["ps"][:])
            filler.append((60, cp))

        def enq_proj_v(tcn):
            box = {}
            for tb4 in range(4):
                for ck2 in range(0, CK, 2):
                    def mm(tb4=tb4, ck2=ck2):
                        if tb4 == 0 and ck2 == 0:
                            box["ps"] = big.tile([128, 4, 256], F32, tag="big",
                                                 name="vps")
                        for ck in (ck2, ck2 + 1):
                            nc.tensor.matmul(
                                box["ps"][:, tb4, :],
                                lhsT=xts[tcn][:, ck, ts(tb4, 128)],
                                rhs=wv_sb[:, ck, :],
                                start=(ck == 0), stop=(ck == CK - 1),
                            )
                    filler.append((215, mm))
            def cp():
                nc.vector.tensor_copy(
                    v65[:, :, 4 * tcn:4 * tcn + 4, 0:64],
                    box["ps"][:].rearrange("p tb (h a) -> p h tb a", h=HPC),
                )
            filler.append((60, cp))

        def attn_scores(tcn, h):
            """S^T + exp + mask for all s-blocks of (tcn, h); returns pb tiles."""
            nsb = 4 * tcn + 4
            p0, p1 = 64 * (h // 2), 64 * (h // 2) + 64
            jh = h % 2
            # final head: masks on Pool so the tail's AV chain never queues
            # behind DVE staging copies
            mask_eng = (nc.gpsimd if (tcn == TC - 1 and h == HSEQ[-1])
                        else nc.vector)
            budget = 700
            pbs = []
            for sb2 in range(0, nsb, 2):
                sps = big.tile([128, 2, 512], F32, tag="big", name="sps")
                for jj in range(2):
                    sb = sb2 + jj
                    if FP8_S:
                        # DoubleRow pair = (head slot jh, zeros slot 2)
                        nc.tensor.matmul(
                            sps[:, jj, :],
                            lhsT=ks[p0:p1, jh:3:2 - jh, ts(sb, 128)],
                            rhs=qs[p0:p1, jh:3:2 - jh, ts(tcn, 512)],
                            start=True, stop=True, perf_mode=DR,
                        )
                    else:
                        nc.tensor.matmul(
                            sps[:, jj, :],
                            lhsT=ks[p0:p1, jh, ts(sb, 128)],
                            rhs=qs[p0:p1, jh, ts(tcn, 512)],
                            start=True, stop=True,
                        )
                pb = pbp.tile([128, 2, 512], BF16, tag="pb", name="pb")
                d0 = sb2 - 4 * tcn
                if d0 < 2:
                    nc.scalar.activation(pb[:], sps[:], AF.Exp, scale=SCALE)
                else:
                    # second diagonal pair: exp only the live tail of each block
                    for jj in range(2):
                        d = d0 + jj
                        nc.gpsimd.memset(pb[:, jj, 0:128 * d], 0.0)
                        nc.scalar.activation(
                            pb[:, jj, 128 * d:512], sps[:, jj, 128 * d:512],
                            AF.Exp, scale=SCALE,
                        )
                for jj in range(2):
                    d = sb2 + jj - 4 * tcn
                    if d >= 0:
                        if d0 < 2:
                            w = 128 * (d + 1)
                            mask_eng.tensor_mul(
                                pb[:, jj, 0:w], pb[:, jj, 0:w], mask_sb[:, d, 0:w])
                        else:
                            mask_eng.tensor_mul(
                                pb[:, jj, 128 * d:128 * (d + 1)],
                                pb[:, jj, 128 * d:128 * (d + 1)],
                                mask_sb[:, d, 128 * d:128 * (d + 1)])
                pbs.append(pb)
                drain(budget)
            return pbs

        def attn_av(tcn, h, pbs, ynsb):
            """A@V (y natural; sequential per-t-block accumulation groups -
            PSUM start zeroing is bank-granular) + normalize."""
            ynat = ynp.tile([128, 4, 65], F32, tag="yn", name="ynat")
            for tb4 in range(4):
                last = 4 * tcn + tb4
                for sb in range(last + 1):
                    nc.tensor.matmul(
                        ynat[:, tb4, :],
                        lhsT=pbs[sb // 2][:, sb % 2, ts(tb4, 128)],
                        rhs=v65[:, h, sb, :],
                        start=(sb == 0), stop=(sb == last),
                    )
            rb = rbp.tile([128, 4], F32, tag="rb", name="rb")
            nc.vector.reciprocal(rb[:], ynat[:, :, 64])
            nc.vector.tensor_mul(
                ynsb[:, :, 64 * h:64 * h + 64],
                ynat[:, :, 0:64],
                rb[:].unsqueeze(-1).broadcast_to((128, 4, 64)),
            )

        def enq_transpose(tcn, ynsb, yt):
            box = {}
            for tb4 in range(4):
                def mm(tb4=tb4):
                    if tb4 == 0:
                        box["ps"] = big.tile([128, 1024], F32, tag="big",
                                             name="ytps")
                        box["v"] = box["ps"].bitcast(BF16)[:, 0:1024].rearrange(
                            "p (cl tb t) -> p cl tb t", cl=2, tb=4)
                    for cl in range(2):
                        nc.tensor.transpose(
                            box["v"][:, cl, tb4, :],
                            ynsb[:, tb4, ts(cl, 128)],
                            ident_sb[:],
                        )
                filler.append((110, mm))
            def cp():
                nc.vector.tensor_copy(yt[:], box["v"][:])
            filler.append((60, cp))

        def oproj_mms(ops, yt, tb4):
            for cc in range(2):
                for cl in range(2):
                    nc.tensor.matmul(
                        ops[:, cc, :],
                        lhsT=yt[:, cl, tb4, :],
                        rhs=wo_sb[:, cl, ts(cc, 512)],
                        start=(cl == 0), stop=(cl == 1),
                    )

        def oproj_out(ops, tcn, tb4, copy_eng=None):
            ob = obp.tile([128, C], F32, tag="ob", name="ob")
            if copy_eng is nc.scalar:
                nc.scalar.activation(
                    ob[:], ops[:].rearrange("p c f -> p (c f)"), AF.Copy)
            else:
                nc.vector.tensor_copy(ob[:], ops[:].rearrange("p c f -> p (c f)"))
            eng = nc.sync if tb4 % 2 == 0 else nc.gpsimd
            eng.dma_start(out_d[4 * tcn + tb4], ob[:])

        def enq_oproj(tcn, yt, tb4):
            box = {}
            for cc in range(2):
                def mm(cc=cc):
                    if cc == 0:
                        box["ps"] = big.tile([128, 2, 512], F32, tag="big",
                                             name="ops")
                    for cl in range(2):
                        nc.tensor.matmul(
                            box["ps"][:, cc, :],
                            lhsT=yt[:, cl, tb4, :],
                            rhs=wo_sb[:, cl, ts(cc, 512)],
                            start=(cl == 0), stop=(cl == 1),
                        )
                filler.append((430, mm))
            filler.append((60, lambda: oproj_out(box["ps"], tcn, tb4)))

        # ---- main software pipeline -----------------------------------------
        # Head order (0,2,1,3): heads 0/2 need only the j0 slot of q/k, so
        # attention starts right after the j0 projections of a chunk; j1
        # projections, next-chunk projections and the previous chunk's output
        # projection drip in as filler between score pairs.
        HSEQ = (0, 2, 1, 3)
        # startup: chunk-0 j0 projections issued directly (critical path)
        for w_sb, dst in ((wq_sb, qs), (wk_sb, ks)):
            ps = big.tile([128, 512], F32, tag="big", name="qkps")
            for ck in range(CK):
                nc.tensor.matmul(
                    ps[:], lhsT=w_sb[:, 0, ck, :], rhs=xts[0][:, ck, :],
                    start=(ck == 0), stop=(ck == CK - 1),
                )
            nc.vector.tensor_copy(dst[:, 0, ts(0, 512)], ps[:])

        ynsbs = [None] * TC
        yts = [None] * TC
        for tcn in range(TC):
            ynsb = ynsbp.tile([128, 4, 256], BF16, tag="yn", name=f"ynsb{tcn}")
            ynsbs[tcn] = ynsb
            if tcn > 0:
                drain_until(f"j0-{tcn}")
            enq_proj_qk_j(tcn, 1, wq_sb, qs)
            enq_proj_qk_j(tcn, 1, wk_sb, ks)
            filler.append((None, f"j1-{tcn}"))
            if tcn > 0:
                yts[tcn - 1] = ytsbp.tile([128, 2, 4, 128], BF16, tag="yt",
                                          name="yt")
                enq_transpose(tcn - 1, ynsbs[tcn - 1], yts[tcn - 1])
            else:
                enq_proj_v(0)
                filler.append((None, "v-0"))

            pend = []
            for k, h in enumerate(HSEQ):
                if k == 2:
                    drain_until(f"j1-{tcn}")
                pbs = attn_scores(tcn, h)
                pend.append((h, pbs))
                if k == 2:
                    drain_until(f"v-{tcn}")
                    attn_av(tcn, *pend.pop(0), ynsb)   # AV(h0)
                elif k == 3:
                    attn_av(tcn, *pend.pop(0), ynsb)   # AV(h2)
                    attn_av(tcn, *pend.pop(0), ynsb)   # AV(h1)
                if k == 0 and tcn < TC - 1:
                    load_x(tcn + 1)
                    enq_proj_qk_j(tcn + 1, 0, wq_sb, qs)
                    enq_proj_qk_j(tcn + 1, 0, wk_sb, ks)
                    filler.append((None, f"j0-{tcn + 1}"))
                    enq_proj_v(tcn + 1)
                    filler.append((None, f"v-{tcn + 1}"))
                elif k == 1 and tcn >= 2:
                    # output projections are deferred one extra chunk into the
                    # Act-bound phase where the PE has slack
                    for m in ([0] if tcn == 2 else [1, 2]):
                        for tb4 in range(4):
                            enq_oproj(m, yts[m], tb4)
                elif k == 3 and tcn == TC - 1:
                    # pre-issue cl0 transposes of the final chunk (heads 0/1
                    # normalized by now)
                    fin_ytps = big.tile([128, 1024], F32, tag="big",
                                        name="ytps")
                    fin_ytv = fin_ytps.bitcast(BF16)[:, 0:1024].rearrange(
                        "p (cl tb t) -> p cl tb t", cl=2, tb=4)
                    for tb4 in range(4):
                        nc.tensor.transpose(
                            fin_ytv[:, 0, tb4, :], ynsb[:, tb4, 0:128],
                            ident_sb[:])
            if tcn < TC - 1:
                attn_av(tcn, *pend.pop(0), ynsb)       # AV(h3)
        h_fin, pbs_fin = pend.pop(0)
        drain()
        # final chunk tail: per-t-block AV -> normalize -> transpose ->
        # output projection pipeline (copies alternate DVE/Act; the last
        # block's staging and DMA are split across engines/queues)
        n, h = TC - 1, h_fin
        ynsb = ynsbs[n]
        yt = ytsbp.tile([128, 2, 4, 128], BF16, tag="yt", name="yt")
        ynat = ynp.tile([128, 4, 65], F32, tag="yn", name="ynat")
        rb = rbp.tile([128, 4], F32, tag="rb", name="rb")
        for tb4 in range(4):
            last = 4 * n + tb4
            for sb in range(last + 1):
                nc.tensor.matmul(
                    ynat[:, tb4, :],
                    lhsT=pbs_fin[sb // 2][:, sb % 2, ts(tb4, 128)],
                    rhs=v65[:, h, sb, :],
                    start=(sb == 0), stop=(sb == last),
                )
            nc.vector.reciprocal(rb[:, tb4:tb4 + 1], ynat[:, tb4, 64:65])
            nc.vector.tensor_mul(
                ynsb[:, tb4, 64 * h:64 * h + 64],
                ynat[:, tb4, 0:64],
                rb[:, tb4:tb4 + 1].broadcast_to((128, 64)),
            )
            nc.tensor.transpose(
                fin_ytv[:, 1, tb4, :], ynsb[:, tb4, ts(1, 128)], ident_sb[:])
            if tb4 % 2:
                nc.scalar.activation(yt[:, :, tb4, :], fin_ytv[:, :, tb4, :],
                                     AF.Copy)
            else:
                nc.vector.tensor_copy(yt[:, :, tb4, :], fin_ytv[:, :, tb4, :])
        for tb4 in range(4):
            ops = big.tile([128, 2, 512], F32, tag="big", name="ops")
            oproj_mms(ops, yt, tb4)
            if tb4 < 3:
                oproj_out(ops, n, tb4,
                          copy_eng=nc.scalar if tb4 % 2 == 0 else None)
            else:
                ob = obp.tile([128, C], F32, tag="ob", name="ob")
                nc.vector.tensor_copy(ob[:, 0:512], ops[:, 0, :])
                nc.scalar.activation(ob[:, 512:1024], ops[:, 1, :], AF.Copy)
                nc.sync.dma_start(out_d[4 * n + tb4, :, 0:512], ob[:, 0:512])
                nc.gpsimd.dma_start(out_d[4 * n + tb4, :, 512:1024],
                                    ob[:, 512:1024])

    nc.compile()
    return nc


_NC = None


def _get_nc():
    global _NC
    if _NC is None:
        _NC = build_nc()
    return _NC


def _mask_arr():
    p = np.arange(128)[:, None, None]
    d = np.arange(4)[None, :, None]
    f = np.arange(512)[None, None, :]
    return (128 * d + p <= f).astype(ml_dtypes.bfloat16)


def _bf16(a):
    return np.ascontiguousarray(np.asarray(a, np.float32).astype(ml_dtypes.bfloat16))


def make_in_maps(x, w_q, w_k, w_v, w_o):
    x = np.asarray(x, dtype=np.float32)
    w_q = np.asarray(w_q, dtype=np.float32)
    w_k = np.asarray(w_k, dtype=np.float32)
    w_v = np.asarray(w_v, dtype=np.float32)
    w_o = np.asarray(w_o, dtype=np.float32)
    mask = np.ascontiguousarray(_mask_arr())
    ident = np.eye(128, dtype=ml_dtypes.bfloat16)
    in_maps = []
    for c in range(NCORES):
        b, g = c // 4, c % 4
        hs = [4 * g + i for i in range(HPC)]
        # xT: [TC, 128, CK, 512] (p-major per chunk)
        xT = x[b].T.reshape(CK, 128, TC, 512).transpose(2, 1, 0, 3)

        def qk_layout(w):
            # [128, 2, CK, 128]: parity-j columns = heads (j, j+2); partition
            # group g holds head 2g+j's 64 a-columns
            per_j = []
            for j in range(2):
                cols = np.concatenate(
                    [w[hs[j]], w[hs[j + 2]]], axis=1)  # [C, 128]
                per_j.append(cols.reshape(CK, 128, 128).transpose(1, 0, 2))
            return np.stack(per_j, axis=1)  # [128, 2, CK, 128]

        wv_a = np.concatenate([w_v[h] for h in hs], axis=1)  # [C, 256]
        wv_a = wv_a.reshape(CK, 128, 256).transpose(1, 0, 2)
        wo_a = w_o[256 * g:256 * (g + 1)].reshape(2, 128, C).transpose(1, 0, 2)
        in_maps.append(dict(
            mask=mask,
            ident=ident,
            xT=_bf16(xT),
            wq=_bf16(qk_layout(w_q)),
            wk=_bf16(qk_layout(w_k)),
            wv=_bf16(wv_a),
            wo=_bf16(wo_a),
        ))
    return in_maps


def gather_out(results):
    acc = [np.zeros((T, C), np.float64) for _ in range(B)]
    for c in range(NCORES):
        acc[c // 4] += results[c]["out"].reshape(T, C).astype(np.float64)
    return np.stack([a.astype(np.float32) for a in acc])


def run(x, w_q, w_k, w_v, w_o, trace=False, **spmd_kwargs):
    nc = _get_nc()
    in_maps = make_in_maps(x, w_q, w_k, w_v, w_o)
    res = run_bass_kernel_spmd(nc, in_maps, list(range(NCORES)), trace=trace,
                               **spmd_kwargs)
    return gather_out(res.results), res


def kernel(x, w_q, w_k, w_v, w_o):
    out, _ = run(x, w_q, w_k, w_v, w_o)
    return out


# revision 58
# speedup vs baseline: 1.4989x; 1.0039x over previous
"""Causal multi-head attention (B=2,T=2048,C=1024,H=16,Ca=64) on 8 trn2 cores.

Sharding: the 32 (batch, head) pairs are split across 8 cores - core c gets
batch b = c//4 and heads [4g, 4g+4) where g = c%4.  Each core computes its
heads' attention plus the partial output projection through its 256-row slice
of w_o; the host sums the 4 partials per batch.

Pipeline (per core), bf16 storage + fp32 PSUM, scores matmul in fp8-e4m3
DoubleRow (2x PE rate):
  - Q/K projections emit a [128=(2 groups x 64a), j, t] layout (head parity
    j within each 64-partition group, plus a zeros slot j=2) so the per-head
    fp8 scores matmul runs in DoubleRow mode - pair (head, zeros) - at
    0.5 cycles/row.
  - Scores are computed transposed (S^T[s,t]) per 512-t chunk; exp on the
    Act engine writes bf16 P^T tiles; diagonal-block triangles are zeroed
    by 0/1 mask multiplies on DVE.
  - A@V uses P^T blocks as the matmul stationary so y lands NATURAL
    [t, a] at only 65 moving rows per (t-block, s-block); the extra ones
    column of V yields the softmax denominators l in column 64.
  - normalize = per-partition reciprocal + broadcast multiply (DVE).
  - y_nat is transposed back via PE-transposes (128 rows each) for the
    output projection, whose [128,1024] psum is staged to SBUF by the Pool
    engine and DMA'd out in fp32.
Chunk-major software pipeline: projections of chunk n+1 and the output
projection of chunk n-1 are interleaved between the 4 heads of chunk n so
the Act engine (exp) never starves.
"""

import math
import sys

import numpy as np
import ml_dtypes

for _p in ("/opt/trn_rl_repo",):
    if _p not in sys.path:
        sys.path.insert(0, _p)

import concourse.bass as bass
from concourse import bacc
import concourse.mybir as mybir
from concourse.bass import ts
from concourse.tile import TileContext
from concourse.bass_utils import run_bass_kernel_spmd
from contextlib import ExitStack

F32 = mybir.dt.float32
BF16 = mybir.dt.bfloat16
FP8 = mybir.dt.float8e4
AF = mybir.ActivationFunctionType
DR = mybir.MatmulPerfMode.DoubleRow

B, T, C = 2, 2048, 1024
H, CA = 16, 64
SCALE = 1.0 / math.sqrt(CA)
NCORES = 8
HPC = 4          # heads per core
TB = T // 128    # 16 t-blocks of 128
TC = T // 512    # 4 t-chunks of 512
CK = C // 128    # 8 c-chunks

FP8_S = True     # fp8-e4m3 DoubleRow scores matmul
QK_DT = FP8 if FP8_S else BF16


def build_nc():
    nc = bacc.Bacc()
    xT_d = nc.declare_dram_parameter("xT", [TC, 128, CK, 512], BF16, isOutput=False)
    wq_d = nc.declare_dram_parameter("wq", [128, 2, CK, 128], BF16, isOutput=False)
    wk_d = nc.declare_dram_parameter("wk", [128, 2, CK, 128], BF16, isOutput=False)
    wv_d = nc.declare_dram_parameter("wv", [128, CK, 256], BF16, isOutput=False)
    wo_d = nc.declare_dram_parameter("wo", [128, 2, C], BF16, isOutput=False)
    mask_d = nc.declare_dram_parameter("mask", [128, 4, 512], BF16, isOutput=False)
    ident_d = nc.declare_dram_parameter("ident", [128, 128], BF16, isOutput=False)
    out_d = nc.declare_dram_parameter("out", [TB, 128, C], F32, isOutput=True)

    with TileContext(nc) as tc, ExitStack() as ctx:
        const = ctx.enter_context(tc.tile_pool(name="const", bufs=1))
        persist = ctx.enter_context(tc.tile_pool(name="persist", bufs=1))
        xp = ctx.enter_context(tc.tile_pool(name="xp", bufs=2))
        pbp = ctx.enter_context(tc.tile_pool(name="pbp", bufs=28))
        ynsbp = ctx.enter_context(tc.tile_pool(name="ynsbp", bufs=2))
        ytsbp = ctx.enter_context(tc.tile_pool(name="ytsbp", bufs=4))
        obp = ctx.enter_context(tc.tile_pool(name="obp", bufs=4))
        rbp = ctx.enter_context(tc.tile_pool(name="rbp", bufs=4))
        big = ctx.enter_context(tc.tile_pool(name="big", bufs=3, space="PSUM"))
        ynp = ctx.enter_context(tc.tile_pool(name="ynp", bufs=2, space="PSUM"))

        # ---- constant loads; j0 weight halves + x chunk 0 first (critical path)
        wq_sb = const.tile([128, 2, CK, 128], BF16, tag="wq", name="wq_sb")
        wk_sb = const.tile([128, 2, CK, 128], BF16, tag="wk", name="wk_sb")
        wv_sb = const.tile([128, CK, 256], BF16, tag="wv", name="wv_sb")
        wo_sb = const.tile([128, 2, C], BF16, tag="wo", name="wo_sb")
        mask_sb = const.tile([128, 4, 512], BF16, tag="mask", name="mask_sb")
        ident_sb = const.tile([128, 128], BF16, tag="ident", name="ident_sb")

        # persistent activations; q/k: [128=(grp, a), j(head parity; 2=zeros), t]
        qs = persist.tile([128, 3, T], QK_DT, tag="q", name="qs")
        ks = persist.tile([128, 3, T], QK_DT, tag="k", name="ks")
        v65 = persist.tile([128, HPC, TB, 65], BF16, tag="v", name="v65")

        xts = [None] * TC

        def load_x(tcn, eng4=False):
            t = xp.tile([128, CK, 512], BF16, tag="xt", name=f"xt{tcn}")
            if eng4:
                nc.sync.dma_start(t[:, 0:3, :], xT_d[tcn, :, 0:3, :])
                nc.gpsimd.dma_start(t[:, 3:6, :], xT_d[tcn, :, 3:6, :])
                nc.scalar.dma_start(t[:, 6:8, :], xT_d[tcn, :, 6:8, :])
            else:
                nc.sync.dma_start(t[:, 0:4, :], xT_d[tcn, :, 0:4, :])
                nc.gpsimd.dma_start(t[:, 4:8, :], xT_d[tcn, :, 4:8, :])
            xts[tcn] = t

        nc.sync.dma_start(wq_sb[:, 0], wq_d[:, 0])
        nc.gpsimd.dma_start(wk_sb[:, 0], wk_d[:, 0])
        load_x(0, eng4=True)
        nc.sync.dma_start(wq_sb[:, 1], wq_d[:, 1])
        nc.gpsimd.dma_start(wk_sb[:, 1], wk_d[:, 1])
        if FP8_S:
            nc.gpsimd.memset(qs[:, 2, :], 0.0)
            nc.gpsimd.memset(ks[:, 2, :], 0.0)
        nc.sync.dma_start(wv_sb[:], wv_d[:])
        nc.gpsimd.dma_start(wo_sb[:], wo_d[:])
        nc.sync.dma_start(mask_sb[:], mask_d[:])
        nc.sync.dma_start(ident_sb[:], ident_d[:])
        nc.vector.memset(v65[:, :, :, 64], 1.0)

        # ---- filler machinery ------------------------------------------------
        # Deadline-loose PE work (projections, transposes, output projection)
        # is queued as small steps and drained a few hundred ns at a time
        # between score pairs, so the Act engine (exp) never starves behind a
        # long block of non-score PE work.  Items: (est_ns, fn) or (None, mark).
        from collections import deque
        filler = deque()
        done_marks = set()

        def drain(budget=None):
            while filler:
                est, x = filler[0]
                if est is None:
                    filler.popleft()
                    done_marks.add(x)
                    continue
                if budget is not None and budget < est:
                    return
                filler.popleft()
                x()
                if budget is not None:
                    budget -= est

        def drain_until(mark):
            while mark not in done_marks:
                est, x = filler.popleft()
                if est is None:
                    done_marks.add(x)
                else:
                    x()

        def enq_proj_qk_j(tcn, j, w_sb, dst):
            box = {}
            for ck in range(CK):
                def mm(ck=ck):
                    if ck == 0:
                        box["ps"] = big.tile([128, 512], F32, tag="big",
                                             name="qkps")
                    nc.tensor.matmul(
                        box["ps"][:],
                        lhsT=w_sb[:, j, ck, :],
                        rhs=xts[tcn][:, ck, :],
                        start=(ck == 0), stop=(ck == CK - 1),
                    )
                filler.append((215, mm))
            def cp():
                nc.vector.tensor_copy(dst[:, j, ts(tcn, 512)], box["ps"][:])
            filler.append((60, cp))

        def enq_proj_v(tcn):
            box = {}
            for tb4 in range(4):
                for ck2 in range(0, CK, 2):
                    def mm(tb4=tb4, ck2=ck2):
                        if tb4 == 0 and ck2 == 0:
                            box["ps"] = big.tile([128, 4, 256], F32, tag="big",
                                                 name="vps")
                        for ck in (ck2, ck2 + 1):
                            nc.tensor.matmul(
                                box["ps"][:, tb4, :],
                                lhsT=xts[tcn][:, ck, ts(tb4, 128)],
                                rhs=wv_sb[:, ck, :],
                                start=(ck == 0), stop=(ck == CK - 1),
                            )
                    filler.append((215, mm))
            def cp():
                nc.vector.tensor_copy(
                    v65[:, :, 4 * tcn:4 * tcn + 4, 0:64],
                    box["ps"][:].rearrange("p tb (h a) -> p h tb a", h=HPC),
                )
            filler.append((60, cp))

        def attn_scores(tcn, h):
            """S^T + exp + mask for all s-blocks of (tcn, h); returns pb tiles."""
            nsb = 4 * tcn + 4
            p0, p1 = 64 * (h // 2), 64 * (h // 2) + 64
            jh = h % 2
            # final head: masks on Pool so the tail's AV chain never queues
            # behind DVE staging copies
            is_final = tcn == TC - 1 and h == HSEQ[-1]
            mask_eng = nc.gpsimd if is_final else nc.vector
            budget = 700
            pbs = []
            for sb2 in range(0, nsb, 2):
                sps = big.tile([128, 2, 512], F32, tag="big", name="sps")
                for jj in range(2):
                    sb = sb2 + jj
                    if FP8_S:
                        # DoubleRow pair = (head slot jh, zeros slot 2)
                        nc.tensor.matmul(
                            sps[:, jj, :],
                            lhsT=ks[p0:p1, jh:3:2 - jh, ts(sb, 128)],
                            rhs=qs[p0:p1, jh:3:2 - jh, ts(tcn, 512)],
                            start=True, stop=True, perf_mode=DR,
                        )
                    else:
                        nc.tensor.matmul(
                            sps[:, jj, :],
                            lhsT=ks[p0:p1, jh, ts(sb, 128)],
                            rhs=qs[p0:p1, jh, ts(tcn, 512)],
                            start=True, stop=True,
                        )
                pb = pbp.tile([128, 2, 512], BF16, tag="pb", name="pb")
                d0 = sb2 - 4 * tcn
                if d0 < 2:
                    nc.scalar.activation(pb[:], sps[:], AF.Exp, scale=SCALE)
                else:
                    # second diagonal pair: exp only the live tail of each block
                    for jj in range(2):
                        d = d0 + jj
                        nc.gpsimd.memset(pb[:, jj, 0:128 * d], 0.0)
                        nc.scalar.activation(
                            pb[:, jj, 128 * d:512], sps[:, jj, 128 * d:512],
                            AF.Exp, scale=SCALE,
                        )
                for jj in range(2):
                    d = sb2 + jj - 4 * tcn
                    if d >= 0:
                        if d0 < 2:
                            w = 128 * (d + 1)
                            mask_eng.tensor_mul(
                                pb[:, jj, 0:w], pb[:, jj, 0:w], mask_sb[:, d, 0:w])
                        else:
                            mask_eng.tensor_mul(
                                pb[:, jj, 128 * d:128 * (d + 1)],
                                pb[:, jj, 128 * d:128 * (d + 1)],
                                mask_sb[:, d, 128 * d:128 * (d + 1)])
                pbs.append(pb)
                drain(budget)
            return pbs

        def attn_av(tcn, h, pbs, ynsb):
            """A@V (y natural; sequential per-t-block accumulation groups -
            PSUM start zeroing is bank-granular) + normalize."""
            ynat = ynp.tile([128, 4, 65], F32, tag="yn", name="ynat")
            for tb4 in range(4):
                last = 4 * tcn + tb4
                for sb in range(last + 1):
                    nc.tensor.matmul(
                        ynat[:, tb4, :],
                        lhsT=pbs[sb // 2][:, sb % 2, ts(tb4, 128)],
                        rhs=v65[:, h, sb, :],
                        start=(sb == 0), stop=(sb == last),
                    )
            rb = rbp.tile([128, 4], F32, tag="rb", name="rb")
            nc.vector.reciprocal(rb[:], ynat[:, :, 64])
            nc.vector.tensor_mul(
                ynsb[:, :, 64 * h:64 * h + 64],
                ynat[:, :, 0:64],
                rb[:].unsqueeze(-1).broadcast_to((128, 4, 64)),
            )

        def enq_transpose(tcn, ynsb, yt):
            box = {}
            for tb4 in range(4):
                def mm(tb4=tb4):
                    if tb4 == 0:
                        box["ps"] = big.tile([128, 1024], F32, tag="big",
                                             name="ytps")
                        box["v"] = box["ps"].bitcast(BF16)[:, 0:1024].rearrange(
                            "p (cl tb t) -> p cl tb t", cl=2, tb=4)
                    for cl in range(2):
                        nc.tensor.transpose(
                            box["v"][:, cl, tb4, :],
                            ynsb[:, tb4, ts(cl, 128)],
                            ident_sb[:],
                        )
                filler.append((110, mm))
            def cp():
                nc.vector.tensor_copy(yt[:], box["v"][:])
            filler.append((60, cp))

        def oproj_mms(ops, yt, tb4):
            for cc in range(2):
                for cl in range(2):
                    nc.tensor.matmul(
                        ops[:, cc, :],
                        lhsT=yt[:, cl, tb4, :],
                        rhs=wo_sb[:, cl, ts(cc, 512)],
                        start=(cl == 0), stop=(cl == 1),
                    )

        def oproj_out(ops, tcn, tb4, copy_eng=None):
            ob = obp.tile([128, C], F32, tag="ob", name="ob")
            if copy_eng is nc.scalar:
                nc.scalar.activation(
                    ob[:], ops[:].rearrange("p c f -> p (c f)"), AF.Copy)
            else:
                nc.vector.tensor_copy(ob[:], ops[:].rearrange("p c f -> p (c f)"))
            eng = nc.sync if tb4 % 2 == 0 else nc.gpsimd
            eng.dma_start(out_d[4 * tcn + tb4], ob[:])

        def enq_oproj(tcn, yt, tb4):
            box = {}
            for cc in range(2):
                def mm(cc=cc):
                    if cc == 0:
                        box["ps"] = big.tile([128, 2, 512], F32, tag="big",
                                             name="ops")
                    for cl in range(2):
                        nc.tensor.matmul(
                            box["ps"][:, cc, :],
                            lhsT=yt[:, cl, tb4, :],
                            rhs=wo_sb[:, cl, ts(cc, 512)],
                            start=(cl == 0), stop=(cl == 1),
                        )
                filler.append((430, mm))
            filler.append((60, lambda: oproj_out(box["ps"], tcn, tb4)))

        # ---- main software pipeline -----------------------------------------
        # Head order (0,2,1,3): heads 0/2 need only the j0 slot of q/k, so
        # attention starts right after the j0 projections of a chunk; j1
        # projections, next-chunk projections and the previous chunk's output
        # projection drip in as filler between score pairs.
        HSEQ = (0, 2, 1, 3)
        # startup: chunk-0 j0 projections issued directly (critical path);
        # copies go to different engines so S isn't serialized behind both
        for w_sb, dst, ceng in ((wq_sb, qs, nc.vector), (wk_sb, ks, nc.scalar)):
            ps = big.tile([128, 512], F32, tag="big", name="qkps")
            for ck in range(CK):
                nc.tensor.matmul(
                    ps[:], lhsT=w_sb[:, 0, ck, :], rhs=xts[0][:, ck, :],
                    start=(ck == 0), stop=(ck == CK - 1),
                )
            if ceng is nc.scalar:
                nc.scalar.activation(dst[:, 0, ts(0, 512)], ps[:], AF.Copy)
            else:
                nc.vector.tensor_copy(dst[:, 0, ts(0, 512)], ps[:])

        ynsbs = [None] * TC
        yts = [None] * TC
        for tcn in range(TC):
            ynsb = ynsbp.tile([128, 4, 256], BF16, tag="yn", name=f"ynsb{tcn}")
            ynsbs[tcn] = ynsb
            if tcn > 0:
                drain_until(f"j0-{tcn}")
            enq_proj_qk_j(tcn, 1, wq_sb, qs)
            enq_proj_qk_j(tcn, 1, wk_sb, ks)
            filler.append((None, f"j1-{tcn}"))
            if tcn > 0:
                yts[tcn - 1] = ytsbp.tile([128, 2, 4, 128], BF16, tag="yt",
                                          name="yt")
                enq_transpose(tcn - 1, ynsbs[tcn - 1], yts[tcn - 1])
            else:
                enq_proj_v(0)
                filler.append((None, "v-0"))

            pend = []
            for k, h in enumerate(HSEQ):
                if k == 2:
                    drain_until(f"j1-{tcn}")
                pbs = attn_scores(tcn, h)
                pend.append((h, pbs))
                if k == 2:
                    drain_until(f"v-{tcn}")
                    attn_av(tcn, *pend.pop(0), ynsb)   # AV(h0)
                elif k == 3:
                    attn_av(tcn, *pend.pop(0), ynsb)   # AV(h2)
                    attn_av(tcn, *pend.pop(0), ynsb)   # AV(h1)
                if k == 0 and tcn < TC - 1:
                    load_x(tcn + 1)
                    enq_proj_qk_j(tcn + 1, 0, wq_sb, qs)
                    enq_proj_qk_j(tcn + 1, 0, wk_sb, ks)
                    filler.append((None, f"j0-{tcn + 1}"))
                    enq_proj_v(tcn + 1)
                    filler.append((None, f"v-{tcn + 1}"))
                elif k == 1 and tcn >= 2:
                    # output projections are deferred one extra chunk into the
                    # Act-bound phase where the PE has slack
                    for m in ([0] if tcn == 2 else [1, 2]):
                        for tb4 in range(4):
                            enq_oproj(m, yts[m], tb4)
                elif k == 3 and tcn == TC - 1:
                    # pre-issue cl0 transposes of the final chunk (heads 0/1
                    # normalized by now)
                    fin_ytps = big.tile([128, 1024], F32, tag="big",
                                        name="ytps")
                    fin_ytv = fin_ytps.bitcast(BF16)[:, 0:1024].rearrange(
                        "p (cl tb t) -> p cl tb t", cl=2, tb=4)
                    for tb4 in range(4):
                        nc.tensor.transpose(
                            fin_ytv[:, 0, tb4, :], ynsb[:, tb4, 0:128],
                            ident_sb[:])
            if tcn < TC - 1:
                attn_av(tcn, *pend.pop(0), ynsb)       # AV(h3)
        h_fin, pbs_fin = pend.pop(0)
        # final chunk tail: per-t-block AV -> normalize -> transpose ->
        # output projection pipeline (copies alternate DVE/Act; the last
        # block's staging and DMA are split across engines/queues)
        n, h = TC - 1, h_fin
        ynsb = ynsbs[n]
        yt = ytsbp.tile([128, 2, 4, 128], BF16, tag="yt", name="yt")
        ynat = ynp.tile([128, 4, 65], F32, tag="yn", name="ynat")
        for tb4 in range(4):
            last = 4 * n + tb4
            for sb in range(last + 1):
                nc.tensor.matmul(
                    ynat[:, tb4, :],
                    lhsT=pbs_fin[sb // 2][:, sb % 2, ts(tb4, 128)],
                    rhs=v65[:, h, sb, :],
                    start=(sb == 0), stop=(sb == last),
                )
            rbt = rbp.tile([128, 1], F32, tag="rbt", name="rbt")
            nc.vector.reciprocal(rbt[:], ynat[:, tb4, 64:65])
            nc.vector.tensor_mul(
                ynsb[:, tb4, 64 * h:64 * h + 64],
                ynat[:, tb4, 0:64],
                rbt[:].broadcast_to((128, 64)),
            )
            nc.tensor.transpose(
                fin_ytv[:, 1, tb4, :], ynsb[:, tb4, ts(1, 128)], ident_sb[:])
            if tb4 % 2:
                nc.scalar.activation(yt[:, :, tb4, :], fin_ytv[:, :, tb4, :],
                                     AF.Copy)
            else:
                nc.vector.tensor_copy(yt[:, :, tb4, :], fin_ytv[:, :, tb4, :])
        for tb4 in range(4):
            ops = big.tile([128, 2, 512], F32, tag="big", name="ops")
            oproj_mms(ops, yt, tb4)
            if tb4 < 3:
                oproj_out(ops, n, tb4,
                          copy_eng=nc.scalar if tb4 % 2 == 0 else None)
            else:
                ob = obp.tile([128, C], F32, tag="ob", name="ob")
                nc.vector.tensor_copy(ob[:, 0:512], ops[:, 0, :])
                nc.scalar.activation(ob[:, 512:1024], ops[:, 1, :], AF.Copy)
                nc.sync.dma_start(out_d[4 * n + tb4, :, 0:512], ob[:, 0:512])
                nc.gpsimd.dma_start(out_d[4 * n + tb4, :, 512:1024],
                                    ob[:, 512:1024])
        drain()

    nc.compile()
    return nc


_NC = None


def _get_nc():
    global _NC
    if _NC is None:
        _NC = build_nc()
    return _NC


def _mask_arr():
    p = np.arange(128)[:, None, None]
    d = np.arange(4)[None, :, None]
    f = np.arange(512)[None, None, :]
    return (128 * d + p <= f).astype(ml_dtypes.bfloat16)


def _bf16(a):
    return np.ascontiguousarray(np.asarray(a, np.float32).astype(ml_dtypes.bfloat16))


def make_in_maps(x, w_q, w_k, w_v, w_o):
    x = np.asarray(x, dtype=np.float32)
    w_q = np.asarray(w_q, dtype=np.float32)
    w_k = np.asarray(w_k, dtype=np.float32)
    w_v = np.asarray(w_v, dtype=np.float32)
    w_o = np.asarray(w_o, dtype=np.float32)
    mask = np.ascontiguousarray(_mask_arr())
    ident = np.eye(128, dtype=ml_dtypes.bfloat16)
    in_maps = []
    for c in range(NCORES):
        b, g = c // 4, c % 4
        hs = [4 * g + i for i in range(HPC)]
        # xT: [TC, 128, CK, 512] (p-major per chunk)
        xT = x[b].T.reshape(CK, 128, TC, 512).transpose(2, 1, 0, 3)

        def qk_layout(w):
            # [128, 2, CK, 128]: parity-j columns = heads (j, j+2); partition
            # group g holds head 2g+j's 64 a-columns
            per_j = []
            for j in range(2):
                cols = np.concatenate(
                    [w[hs[j]], w[hs[j + 2]]], axis=1)  # [C, 128]
                per_j.append(cols.reshape(CK, 128, 128).transpose(1, 0, 2))
            return np.stack(per_j, axis=1)  # [128, 2, CK, 128]

        wv_a = np.concatenate([w_v[h] for h in hs], axis=1)  # [C, 256]
        wv_a = wv_a.reshape(CK, 128, 256).transpose(1, 0, 2)
        wo_a = w_o[256 * g:256 * (g + 1)].reshape(2, 128, C).transpose(1, 0, 2)
        in_maps.append(dict(
            mask=mask,
            ident=ident,
            xT=_bf16(xT),
            wq=_bf16(qk_layout(w_q)),
            wk=_bf16(qk_layout(w_k)),
            wv=_bf16(wv_a),
            wo=_bf16(wo_a),
        ))
    return in_maps


def gather_out(results):
    acc = [np.zeros((T, C), np.float64) for _ in range(B)]
    for c in range(NCORES):
        acc[c // 4] += results[c]["out"].reshape(T, C).astype(np.float64)
    return np.stack([a.astype(np.float32) for a in acc])


def run(x, w_q, w_k, w_v, w_o, trace=False, **spmd_kwargs):
    nc = _get_nc()
    in_maps = make_in_maps(x, w_q, w_k, w_v, w_o)
    res = run_bass_kernel_spmd(nc, in_maps, list(range(NCORES)), trace=trace,
                               **spmd_kwargs)
    return gather_out(res.results), res


def kernel(x, w_q, w_k, w_v, w_o):
    out, _ = run(x, w_q, w_k, w_v, w_o)
    return out
